# revision 37
# baseline (speedup 1.0000x reference)
"""Trainium2 Bass kernel for nn_GCNClassifier (dense transformer w/ soft attention pooling).

Contract: kernel(**inputs) takes FULL unsharded inputs (as produced by
setup_inputs()) and returns the full output tuple (output[B,T,D], att[B,T]).
Internally: data-parallel over batch across 8 NeuronCores (2 examples/core).

Per-example device pipeline (bf16 matmul path, fp32 for the scalar chain):
  1. embedding gather (indirect DMA, words -> bf16 emb rows) into t-major g tiles;
     small-table features are host-gathered and streamed as one dense bf16 tensor.
  2. PE transposes g -> d-major layout g_d [500(+ones row), T], bf16.
  3. K/Q/V projections in bf16 (biases folded in via extra contraction row).
  4. subj max-pool / q / t / k-softmax / c / m chain (small fp32 vector-matmuls).
  5. attention in S^T layout: ST[s,t] = key[t].query[s]; exp on ACT (bf16 out);
     A@V with a ones column appended to V giving the scaled-softmax denominator
     for free; unscaled softmax row-sums + diagonal via ones-vector matmuls
     (att = (1 - diag(softmax(S)))/scale). All accumulation in fp32 PSUM.
"""

import numpy as np

B, T, D = 16, 1024, 500
VOCAB, EMB = 50000, 300
NCORES = 8
BLOC = B // NCORES  # 2 examples per core
SCALE = float(np.sqrt(500.0))
NEG = 1e12

_CACHE = {}


def _build_bass():
    from contextlib import ExitStack

    import concourse.bacc as bacc
    import concourse.bass as bass
    import concourse.tile as tile
    from concourse import mybir
    from concourse.masks import make_identity

    f32 = mybir.dt.float32
    bf16 = mybir.dt.bfloat16
    i32 = mybir.dt.int32
    AF = mybir.ActivationFunctionType
    AX = mybir.AxisListType
    OP = mybir.AluOpType

    nc = bacc.Bacc(None, target_bir_lowering=False, debug=False)

    # ---- DRAM I/O ----
    words_d = nc.dram_tensor("words", [BLOC, 128, 8], i32, kind="ExternalInput")
    rest_d = nc.dram_tensor("rest", [BLOC, T, 200], bf16, kind="ExternalInput")
    pen_d = nc.dram_tensor("pen", [BLOC, T], bf16, kind="ExternalInput")
    emb_d = nc.dram_tensor("emb", [VOCAB, EMB], bf16, kind="ExternalInput")
    karr_d = nc.dram_tensor("karr", [126, 4, 500], bf16, kind="ExternalInput")
    qarr_d = nc.dram_tensor("qarr", [126, 4, 500], bf16, kind="ExternalInput")
    varr_d = nc.dram_tensor("varr", [126, 4, 500], bf16, kind="ExternalInput")
    wq_d = nc.dram_tensor("wq", [125, 8, 500], bf16, kind="ExternalInput")
    wc_d = nc.dram_tensor("wc", [125, 4, 500], bf16, kind="ExternalInput")
    wm_d = nc.dram_tensor("wm", [125, 12, 500], bf16, kind="ExternalInput")
    wk_d = nc.dram_tensor("wk", [125, 4], f32, kind="ExternalInput")
    bq_d = nc.dram_tensor("bq", [1, 500], f32, kind="ExternalInput")
    bc_d = nc.dram_tensor("bc", [1, 500], f32, kind="ExternalInput")
    bm_d = nc.dram_tensor("bm", [1, 500], f32, kind="ExternalInput")
    out_d = nc.dram_tensor("out", [BLOC, T, D], f32, kind="ExternalOutput")
    att_d = nc.dram_tensor("att", [BLOC, T], f32, kind="ExternalOutput")

    with tile.TileContext(nc) as tc, ExitStack() as ctx:
        sb = ctx.enter_context(tc.tile_pool(name="sb", bufs=1))
        ps = ctx.enter_context(tc.tile_pool(name="ps", bufs=1, space="PSUM"))

        # ---- constants / weights (loaded once) ----
        ident = sb.tile([128, 128], f32, name="ident", tag="ident", bufs=1)
        make_identity(nc, ident[:])
        ident_bf = sb.tile([128, 128], bf16, name="ident_bf", tag="ident_bf", bufs=1)
        make_identity(nc, ident_bf[:])
        ones_col = sb.tile([128, 1], bf16, name="ones_col", tag="ones_col", bufs=1)
        nc.vector.memset(ones_col[:], 1.0)
        ones_row = sb.tile([1, 128], f32, name="ones_row", tag="ones_row", bufs=1)
        nc.vector.memset(ones_row[:], 1.0)
        ones_row_bf = sb.tile([1, 128], bf16, name="ones_row_bf", tag="ones_row_bf", bufs=1)
        nc.vector.memset(ones_row_bf[:], 1.0)

        def emit_weights():
            karr = sb.tile([126, 4, 500], bf16, name="karr", tag="karr", bufs=1)
            nc.sync.dma_start(karr[:], karr_d[:])
            qarr = sb.tile([126, 4, 500], bf16, name="qarr", tag="qarr", bufs=1)
            nc.sync.dma_start(qarr[:], qarr_d[:])
            varr = sb.tile([126, 4, 500], bf16, name="varr", tag="varr", bufs=1)
            nc.sync.dma_start(varr[:], varr_d[:])
            wq = sb.tile([125, 8, 500], bf16, name="wq", tag="wq", bufs=1)
            nc.sync.dma_start(wq[:], wq_d[:])
            wc = sb.tile([125, 4, 500], bf16, name="wc", tag="wc", bufs=1)
            nc.sync.dma_start(wc[:], wc_d[:])
            wm = sb.tile([125, 12, 500], bf16, name="wm", tag="wm", bufs=1)
            nc.sync.dma_start(wm[:], wm_d[:])
            wk = sb.tile([125, 4], f32, name="wk", tag="wk", bufs=1)
            nc.sync.dma_start(wk[:], wk_d[:])
            bq = sb.tile([1, 500], f32, name="bq", tag="bq", bufs=1)
            nc.sync.dma_start(bq[:], bq_d[:])
            bc = sb.tile([1, 500], f32, name="bc", tag="bc", bufs=1)
            nc.sync.dma_start(bc[:], bc_d[:])
            bm = sb.tile([1, 500], f32, name="bm", tag="bm", bufs=1)
            nc.sync.dma_start(bm[:], bm_d[:])

            return karr, qarr, varr, wq, wc, wm, wk, bq, bc, bm

        # ================= woven per-example pipeline =================
        # Per example: prep (gather+transpose -> g_d, projections), then
        # attention halves. The serial q/t/k/c/m chain and the NEXT example's
        # gather are woven into the attention i-loop so the PE stream stays
        # dense (HAM stays warm). The attention output PSUM is released with
        # cheap copies (partA); normalization/m-multiply (partB) runs off the
        # critical path once the chain's m_b is ready.
        st_ = [dict() for _ in range(BLOC)]

        def emit_gd_init(e):
            s = st_[e]
            s["g_d"] = sb.tile([128, 4, T], bf16, name=f"g_d{e}", tag="g_d", bufs=2)
            nc.vector.memset(s["g_d"][96:128, :, :], 1.0)

        def emit_gather_init(e):
            s = st_[e]
            widx = sb.tile([128, 8], i32, name=f"widx{e}", tag="widx", bufs=2)
            nc.sync.dma_start(widx[:], words_d[e, :, :])
            s["widx"] = widx
            g_t = sb.tile([128, 8, 500], bf16, name=f"g_t{e}", tag="g_t", bufs=2)
            nc.sync.dma_start(
                g_t[:, :, EMB:500],
                rest_d[e].rearrange("(c p) f -> p c f", p=128),
            )
            s["g_t"] = g_t

        def emit_gather_chunk(e, tj):
            s = st_[e]
            t0 = tj * 128
            g_t = s["g_t"]
            nc.gpsimd.indirect_dma_start(
                out=g_t[:, tj, 0:EMB],
                out_offset=None,
                in_=emb_d[:],
                in_offset=bass.IndirectOffsetOnAxis(ap=s["widx"][:, tj : tj + 1], axis=0),
            )
            for c in range(4):
                tp = ps.tile([125, 128], bf16, name=f"tp{e}_{tj}_{c}", tag="pm", bufs=2)
                nc.tensor.transpose(tp[:], g_t[:, tj, c * 125 : (c + 1) * 125], ident_bf[:])
                nc.vector.tensor_copy(s["g_d"][0:125, c, t0 : t0 + 128], tp[:])

        def emit_proj_init(e):
            s = st_[e]
            s["keyT"] = sb.tile([125, 4, T], bf16, name=f"keyT{e}", tag="keyT", bufs=2)
            s["queryT"] = sb.tile([125, 4, T], bf16, name=f"queryT{e}", tag="queryT", bufs=2)
            s["value"] = sb.tile([128, 8, 501], bf16, name=f"value{e}", tag="value", bufs=2)

        def emit_proj_kq(e, h):
            s = st_[e]
            g_d = s["g_d"]
            for arr, dkey in ((karr, "keyT"), (qarr, "queryT")):
                for mc in range(4):
                    pp = ps.tile([125, 512], f32, name=f"pp{e}_{mc}_{h}", tag="pm", bufs=2)
                    for kk in range(4):
                        nc.tensor.matmul(
                            pp[:],
                            lhsT=arr[:, kk, mc * 125 : (mc + 1) * 125],
                            rhs=g_d[0:126, kk, h * 512 : (h + 1) * 512],
                            start=(kk == 0),
                            stop=(kk == 3),
                        )
                    nc.scalar.copy(s[dkey][:, mc, h * 512 : (h + 1) * 512], pp[:])

        def emit_value(e, tj):
            s = st_[e]
            g_d = s["g_d"]
            pv = ps.tile([128, 500], f32, name=f"pv{e}_{tj}", tag="pm", bufs=2)
            for kk in range(4):
                nc.tensor.matmul(
                    pv[:],
                    lhsT=g_d[0:126, kk, tj * 128 : (tj + 1) * 128],
                    rhs=varr[:, kk, :],
                    start=(kk == 0),
                    stop=(kk == 3),
                )
            nc.scalar.copy(s["value"][:, tj, 0:500], pv[:])
            nc.vector.memset(s["value"][:, tj, 500:501], 1.0)

        def emit_warmup(n):
            # keep the PE HAM activity monitor busy during DMA-bound stretches
            wu = ps.tile([128, 128], f32, name="wu", tag="rows", bufs=2)
            for _ in range(n):
                nc.tensor.matmul(wu[:], lhsT=ident_bf[:], rhs=ident_bf[:], start=True, stop=True)

        def chain_pieces(e):
            s = st_[e]
            g_d = s["g_d"]

            def p0():  # penalty bcast + subj max-pool (first half)
                s["pen_b"] = sb.tile([128, T], bf16, name=f"pen_b{e}", tag="pen_b", bufs=2)
                nc.sync.dma_start(s["pen_b"][:], pen_d[e : e + 1, :].to_broadcast([128, T]))
                s["subj_col"] = sb.tile([125, 4], bf16, name=f"subj_col{e}", tag="subj_col", bufs=2)
                for c in range(2):
                    tmp = sb.tile([125, T], bf16, name=f"tmpm{e}_{c}", tag="tmpm", bufs=2)
                    nc.vector.tensor_tensor(out=tmp[:], in0=g_d[0:125, c, :], in1=s["pen_b"][0:125, :], op=OP.min)
                    nc.vector.reduce_max(s["subj_col"][:, c : c + 1], tmp[:], axis=AX.X)

            def p1():  # subj max-pool (second half)
                for c in range(2, 4):
                    tmp = sb.tile([125, T], bf16, name=f"tmpm{e}_{c}", tag="tmpm", bufs=2)
                    nc.vector.tensor_tensor(out=tmp[:], in0=g_d[0:125, c, :], in1=s["pen_b"][0:125, :], op=OP.min)
                    nc.vector.reduce_max(s["subj_col"][:, c : c + 1], tmp[:], axis=AX.X)

            def p2():  # q = relu(so @ Wq + bq)
                q_ps = ps.tile([1, 512], f32, name=f"q_ps{e}", tag="rows", bufs=2)
                for kk in range(8):
                    nc.tensor.matmul(
                        q_ps[0:1, 0:500],
                        lhsT=s["subj_col"][:, (kk % 4) : (kk % 4) + 1],
                        rhs=wq[:, kk, :],
                        start=(kk == 0),
                        stop=(kk == 7),
                    )
                s["q_row"] = sb.tile([1, 500], f32, name=f"q_row{e}", tag="q_row", bufs=1)
                nc.vector.tensor_add(s["q_row"][:], q_ps[0:1, 0:500], bq[:])
                nc.scalar.activation(s["q_row"][:], s["q_row"][:], AF.Relu)

            def p3():  # q_row -> q_col
                s["q_col"] = sb.tile([125, 4], bf16, name=f"q_col{e}", tag="q_col", bufs=2)
                for c in range(4):
                    tpv = ps.tile([125, 1], f32, name=f"tpq{e}_{c}", tag="rows", bufs=2)
                    nc.tensor.transpose(tpv[:], s["q_row"][0:1, c * 125 : (c + 1) * 125], ident[0:1, 0:1])
                    nc.vector.tensor_copy(s["q_col"][:, c : c + 1], tpv[:])

            def p4():  # t = relu(q @ Wc + bc), t_row -> t_col
                t_ps = ps.tile([1, 512], f32, name=f"t_ps{e}", tag="rows", bufs=2)
                for kk in range(4):
                    nc.tensor.matmul(
                        t_ps[0:1, 0:500],
                        lhsT=s["q_col"][:, kk : kk + 1],
                        rhs=wc[:, kk, :],
                        start=(kk == 0),
                        stop=(kk == 3),
                    )
                t_row = sb.tile([1, 500], f32, name=f"t_row{e}", tag="t_row", bufs=1)
                nc.vector.tensor_add(t_row[:], t_ps[0:1, 0:500], bc[:])
                nc.scalar.activation(t_row[:], t_row[:], AF.Relu)
                s["t_col"] = sb.tile([125, 4], f32, name=f"t_col{e}", tag="t_col", bufs=2)
                for c in range(4):
                    tpt = ps.tile([125, 1], f32, name=f"tpt{e}_{c}", tag="rows", bufs=2)
                    nc.tensor.transpose(tpt[:], t_row[0:1, c * 125 : (c + 1) * 125], ident[0:1, 0:1])
                    nc.vector.tensor_copy(s["t_col"][:, c : c + 1], tpt[:])

            def p5():  # w_b = t*Wk ; k_logits
                wb_col = sb.tile([125, 4], bf16, name=f"wb_col{e}", tag="wb_col", bufs=2)
                nc.vector.tensor_mul(wb_col[:], s["t_col"][:], wk[:])
                s["klog"] = sb.tile([1, T], f32, name=f"klog{e}", tag="klog", bufs=1)
                for h in range(2):
                    kl_ps = ps.tile([1, 512], f32, name=f"klps{e}_{h}", tag="rows", bufs=2)
                    for kk in range(4):
                        nc.tensor.matmul(
                            kl_ps[0:1, :],
                            lhsT=wb_col[:, kk : kk + 1],
                            rhs=g_d[0:125, kk, h * 512 : (h + 1) * 512],
                            start=(kk == 0),
                            stop=(kk == 3),
                        )
                    nc.vector.tensor_copy(s["klog"][0:1, h * 512 : (h + 1) * 512], kl_ps[0:1, :])

            def p6():  # k = softmax(k_logits), to bf16
                klog = s["klog"]
                kmax = sb.tile([1, 1], f32, name=f"kmax{e}", tag="kmax", bufs=2)
                nc.vector.reduce_max(kmax[:], klog[:], axis=AX.X)
                negmax = sb.tile([1, 1], f32, name=f"negmax{e}", tag="negmax", bufs=2)
                nc.vector.tensor_scalar_mul(negmax[:], kmax[:], -1.0)
                ksum = sb.tile([1, 1], f32, name=f"ksum{e}", tag="ksum", bufs=2)
                nc.scalar.activation(klog[:], klog[:], AF.Exp, bias=negmax[:, 0:1], scale=1.0, accum_out=ksum[:])
                rksum = sb.tile([1, 1], f32, name=f"rksum{e}", tag="rksum", bufs=2)
                nc.vector.reciprocal(rksum[:], ksum[:])
                s["k_row"] = sb.tile([1, T], bf16, name=f"k_rowb{e}", tag="k_rowb", bufs=2)
                nc.vector.tensor_scalar_mul(s["k_row"][:], klog[:], rksum[:, 0:1])

            def p7():  # k broadcast via PE outer product
                s["k_b"] = sb.tile([128, T], bf16, name=f"k_b{e}", tag="k_b", bufs=2)
                for h in range(2):
                    kb_ps = ps.tile([128, 512], f32, name=f"kbps{e}_{h}", tag="pm", bufs=2)
                    nc.tensor.matmul(
                        kb_ps[:],
                        lhsT=ones_row_bf[:],
                        rhs=s["k_row"][0:1, h * 512 : (h + 1) * 512],
                        start=True,
                        stop=True,
                    )
                    nc.vector.tensor_copy(s["k_b"][:, h * 512 : (h + 1) * 512], kb_ps[:])

            def p8():  # c = sum_t k*g
                c32 = sb.tile([125, 4], f32, name=f"c32_{e}", tag="c32", bufs=2)
                for c in range(4):
                    tmpc = sb.tile([125, T], bf16, name=f"tmpc{e}_{c}", tag="tmpm", bufs=2)
                    nc.vector.tensor_mul(tmpc[:], g_d[0:125, c, :], s["k_b"][0:125, :])
                    nc.vector.reduce_sum(c32[:, c : c + 1], tmpc[:], axis=AX.X)
                s["c_col"] = sb.tile([125, 4], bf16, name=f"c_col{e}", tag="c_col", bufs=2)
                nc.vector.tensor_copy(s["c_col"][:], c32[:])

            def p9():  # m = relu([c, subj, subj] @ Wm + bm)
                m_ps = ps.tile([1, 512], f32, name=f"m_ps{e}", tag="rows", bufs=2)
                for kk in range(12):
                    col = s["c_col"] if kk < 4 else s["subj_col"]
                    nc.tensor.matmul(
                        m_ps[0:1, 0:500],
                        lhsT=col[:, (kk % 4) : (kk % 4) + 1],
                        rhs=wm[:, kk, :],
                        start=(kk == 0),
                        stop=(kk == 11),
                    )
                s["m_row"] = sb.tile([1, 500], f32, name=f"m_row{e}", tag="m_row", bufs=1)
                nc.vector.tensor_add(s["m_row"][:], m_ps[0:1, 0:500], bm[:])
                nc.scalar.activation(s["m_row"][:], s["m_row"][:], AF.Relu)

            def p10():  # m broadcast
                s["m_b"] = sb.tile([128, 500], f32, name=f"m_b{e}", tag="m_b", bufs=2)
                mb_ps = ps.tile([128, 500], f32, name=f"mbps{e}", tag="pm", bufs=2)
                nc.tensor.matmul(mb_ps[:], lhsT=ones_row[:], rhs=s["m_row"][0:1, :], start=True, stop=True)
                nc.vector.tensor_copy(s["m_b"][:], mb_ps[:])

            return [p0, p1, p2, p3, p4, p5, p6, p7, p8, p9, p10]

        def emit_attn_iter(e, h, i):
            s = st_[e]
            ts0 = h * 512
            st_ps = ps.tile([128, 512], f32, name=f"st{e}_{h}_{i}", tag="pm", bufs=2)
            for kk in range(4):
                nc.tensor.matmul(
                    st_ps[:],
                    lhsT=s["queryT"][:, kk, i * 128 : (i + 1) * 128],
                    rhs=s["keyT"][:, kk, ts0 : ts0 + 512],
                    start=(kk == 0),
                    stop=(kk == 3),
                )
            exp_s = sb.tile([128, 512], bf16, name=f"exps{e}_{h}_{i}", tag="exp_s", bufs=2)
            nc.scalar.activation(exp_s[:], st_ps[:], AF.Exp, scale=1.0 / SCALE)
            exp_u = sb.tile([128, 512], bf16, name=f"expu{e}_{h}_{i}", tag="exp_u", bufs=8)
            nc.scalar.activation(exp_u[:], st_ps[:], AF.Exp, scale=1.0)
            s.setdefault(("expu", h), []).append(exp_u)
            if i // 4 == h:
                off = (i % 4) * 128
                msk = sb.tile([128, 128], bf16, name=f"msk{e}_{h}_{i}", tag="msk", bufs=2)
                nc.vector.tensor_mul(msk[:], exp_u[:, off : off + 128], ident_bf[:])
                dg_ps = ps.tile([1, 512], f32, name=f"dgps{e}_{h}_{i}", tag="rows", bufs=2)
                nc.tensor.matmul(dg_ps[0:1, 0:128], lhsT=ones_col[:, 0:1], rhs=msk[:], start=True, stop=True)
                nc.vector.tensor_copy(s["diagr"][0:1, i * 128 : (i + 1) * 128], dg_ps[0:1, 0:128])
            for jj in range(4):
                nc.tensor.matmul(
                    s["out_ps"][:, jj, 0:501],
                    lhsT=exp_s[:, jj * 128 : (jj + 1) * 128],
                    rhs=s["value"][:, i, :],
                    start=(i == 0),
                    stop=(i == 7),
                )

        def emit_half_end(e, h):
            s = st_[e]
            ts0 = h * 512
            # unscaled softmax denominator: 8 quick accumulating matmuls
            se_ps = ps.tile([1, 512], f32, name=f"seps{e}_{h}", tag="rows", bufs=2)
            for i, exp_u in enumerate(s[("expu", h)]):
                nc.tensor.matmul(
                    se_ps[0:1, :], lhsT=ones_col[:, 0:1], rhs=exp_u[:], start=(i == 0), stop=(i == 7)
                )
            nc.vector.tensor_copy(s["sumexp"][0:1, ts0 : ts0 + 512], se_ps[0:1, :])
            # partA: release the attention output PSUM (per-bank copies)
            o_raw = sb.tile([128, 4, 501], f32, name=f"o_rawa{e}_{h}", tag="o_raw", bufs=3)
            for jj in range(4):
                eng = nc.scalar if jj % 2 == 0 else nc.vector
                if jj % 2 == 0:
                    nc.scalar.copy(o_raw[:, jj, :], s["out_ps"][:, jj, 0:501])
                else:
                    nc.vector.tensor_copy(o_raw[:, jj, :], s["out_ps"][:, jj, 0:501])
            s.setdefault("o_raw", []).append(o_raw)

        def emit_partB(e, js):
            s = st_[e]
            for j in js:
                o_raw = s["o_raw"][j // 4][:, j % 4, :]
                rec = sb.tile([128, 1], f32, name=f"rec{e}_{j}", tag="rec", bufs=2)
                nc.vector.reciprocal(rec[:], o_raw[:, 500:501])
                o_n = sb.tile([128, 500], f32, name=f"o_n{e}_{j}", tag="o_n", bufs=3)
                nc.vector.tensor_scalar_mul(o_n[:], o_raw[:, 0:500], rec[:, 0:1])
                nc.vector.tensor_mul(o_n[:], o_n[:], s["m_b"][:])
                nc.sync.dma_start(out_d[e, j * 128 : (j + 1) * 128, :], o_n[:])

        def emit_att_assembly(e, h):
            s = st_[e]
            sl = slice(h * 512, (h + 1) * 512)
            se = s["sumexp"][0:1, sl]
            dg = s["diagr"][0:1, sl]
            nc.vector.reciprocal(se, se)
            nc.vector.tensor_mul(dg, dg, se)
            nc.vector.tensor_scalar(
                out=dg,
                in0=dg,
                scalar1=-1.0 / SCALE,
                scalar2=1.0 / SCALE,
                op0=mybir.AluOpType.mult,
                op1=mybir.AluOpType.add,
            )
            nc.sync.dma_start(att_d[e : e + 1, sl], dg)

        # ---- pipeline ----
        emit_gd_init(0)
        emit_proj_init(0)
        emit_gather_init(0)
        karr, qarr, varr, wq, wc, wm, wk, bq, bc, bm = emit_weights()
        emit_warmup(24)
        for tj in range(8):
            emit_gather_chunk(0, tj)
            emit_value(0, tj)
            emit_warmup(8)
            if tj == 3:
                emit_proj_kq(0, 0)
        emit_proj_kq(0, 1)
        for e in range(BLOC):
            s = st_[e]
            nxt = e + 1 < BLOC
            s["sumexp"] = sb.tile([1, T], f32, name=f"sumexp{e}", tag="sumexp", bufs=1)
            s["diagr"] = sb.tile([1, T], f32, name=f"diagr{e}", tag="diagr", bufs=1)
            pieces = chain_pieces(e)
            # h = 0: weave chain pieces 0..7
            s["out_ps"] = ps.tile([128, 4, 512], f32, name=f"out_ps{e}_0", tag="po", bufs=1)
            for i in range(8):
                emit_attn_iter(e, 0, i)
                pieces[i]()
            emit_half_end(e, 0)
            emit_att_assembly(e, 0)
            # h = 1: weave remaining chain pieces, next example's prep, partB
            s["out_ps"] = ps.tile([128, 4, 512], f32, name=f"out_ps{e}_1", tag="po", bufs=1)
            if nxt:
                emit_gd_init(e + 1)
                emit_proj_init(e + 1)
                emit_gather_init(e + 1)
            for i in range(8):
                emit_attn_iter(e, 1, i)
                if i < 3:
                    pieces[8 + i]()
                if nxt:
                    emit_gather_chunk(e + 1, i)
                    emit_value(e + 1, i)
                    if i == 4:
                        emit_proj_kq(e + 1, 0)
                elif 3 <= i < 7:
                    emit_partB(e, [i - 3])
            emit_half_end(e, 1)
            emit_att_assembly(e, 1)
            if nxt:
                emit_proj_kq(e + 1, 1)
                emit_partB(e, range(8))
            else:
                emit_partB(e, range(4, 8))

    nc.finalize()
    return nc


def _prep_host(inputs):
    """Host-side input prep: pack weights into SBUF-friendly layouts, gather the
    small embedding tables, build per-core input maps."""
    import ml_dtypes

    bf16 = ml_dtypes.bfloat16
    f = lambda k: np.asarray(inputs[k], dtype=np.float32)
    ii = lambda k: np.asarray(inputs[k], dtype=np.int64)

    words = ii("words")
    pos = ii("pos")
    ner = ii("ner")
    subj_pos = ii("subj_pos")
    obj_pos = ii("obj_pos")
    chunks = ii("chunks")
    on_path = ii("on_path")
    dep_feat = f("dep_feat")

    emb_w = f("emb_w")
    pos_w = f("pos_w")
    ner_w = f("ner_w")
    chunk_w = f("chunk_w")
    position_w = f("position_w")

    # rest200: host-gathered small-table features, cols 300..500 of g
    rest = np.concatenate(
        [
            pos_w[pos],                     # 35
            ner_w[ner],                     # 30
            chunk_w[chunks],                # 30
            position_w[subj_pos],           # 30
            position_w[obj_pos],            # 30
            on_path[..., None].astype(np.float32),  # 1
            dep_feat,                       # 44
        ],
        axis=2,
    ).astype(bf16)
    assert rest.shape == (B, T, 200)

    # penalty row for the masked max-pool: min(g, pen) == where(subj_pos!=0, -NEG, g)
    pen = np.where(subj_pos != 0, np.float32(-NEG), np.float32(3e38)).astype(bf16)

    def pack_kqv(w, b):
        # [126, 4, 500]: rows 0..124 of chunk c = W[125c : 125c+125]; row 125 of
        # chunk 0 = bias (multiplied by the all-ones row of g_d), else 0.
        arr = np.zeros((4, 126, 500), np.float32)
        w = np.asarray(w, np.float32)
        for c in range(4):
            arr[c, :125] = w[125 * c : 125 * (c + 1)]
        arr[0, 125] = np.asarray(b, np.float32)
        return np.ascontiguousarray(arr.transpose(1, 0, 2).astype(bf16))

    karr = pack_kqv(inputs["K_w"], inputs["K_b"])
    qarr = pack_kqv(inputs["Q_w"], inputs["Q_b"])
    varr = pack_kqv(inputs["V_w"], inputs["V_b"])

    def pack_rhs(w, nchunk):
        w = np.asarray(w, np.float32)
        return np.ascontiguousarray(
            w.reshape(nchunk, 125, 500).transpose(1, 0, 2).astype(bf16)
        )

    wq = pack_rhs(inputs["Wq_w"], 8)
    wc = pack_rhs(np.asarray(inputs["Wc_w"], np.float32)[:500], 4)
    wm = pack_rhs(inputs["Wm_w"], 12)
    wk = np.ascontiguousarray(
        np.asarray(inputs["Wk_w"], np.float32).reshape(4, 125).T
    )  # [125, 4], col c = Wk[125c:125c+125]
    bq = np.asarray(inputs["Wq_b"], np.float32).reshape(1, 500)
    bc = np.asarray(inputs["Wc_b"], np.float32).reshape(1, 500)
    bm = np.asarray(inputs["Wm_b"], np.float32).reshape(1, 500)

    shared = dict(
        emb=np.ascontiguousarray(emb_w.astype(bf16)),
        karr=karr, qarr=qarr, varr=varr,
        wq=wq, wc=wc, wm=wm, wk=wk, bq=bq, bc=bc, bm=bm,
    )
    in_maps = []
    for core in range(NCORES):
        s = slice(core * BLOC, (core + 1) * BLOC)
        m = dict(shared)
        m["words"] = np.ascontiguousarray(words[s].astype(np.int32).reshape(BLOC, 8, 128).transpose(0, 2, 1))
        m["rest"] = np.ascontiguousarray(rest[s])
        m["pen"] = np.ascontiguousarray(pen[s])
        in_maps.append(m)
    return in_maps


def _get_nc():
    if "nc" not in _CACHE:
        _CACHE["nc"] = _build_bass()
    return _CACHE["nc"]


def kernel(trace=False, **inputs):
    from concourse.bass_utils import run_bass_kernel_spmd

    nc = _get_nc()
    in_maps = _prep_host(inputs)
    res = run_bass_kernel_spmd(nc, in_maps, core_ids=list(range(NCORES)), trace=trace)
    results = res.results
    output = np.concatenate([r["out"] for r in results], axis=0)
    att = np.concatenate([r["att"] for r in results], axis=0)
    if trace:
        _CACHE["last_perf"] = res
    return output, att


# revision 38
# speedup vs baseline: 1.0273x; 1.0273x over previous
"""Trainium2 Bass kernel for nn_GCNClassifier (dense transformer w/ soft attention pooling).

Contract: kernel(**inputs) takes FULL unsharded inputs (as produced by
setup_inputs()) and returns the full output tuple (output[B,T,D], att[B,T]).
Internally: data-parallel over batch across 8 NeuronCores (2 examples/core).

Per-example device pipeline (bf16 matmul path, fp32 for the scalar chain):
  1. embedding gather (indirect DMA, words -> bf16 emb rows) into t-major g tiles;
     small-table features are host-gathered and streamed as one dense bf16 tensor.
  2. PE transposes g -> d-major layout g_d [500(+ones row), T], bf16.
  3. K/Q/V projections in bf16 (biases folded in via extra contraction row).
  4. subj max-pool / q / t / k-softmax / c / m chain (small fp32 vector-matmuls).
  5. attention in S^T layout: ST[s,t] = key[t].query[s]; exp on ACT (bf16 out);
     A@V with a ones column appended to V giving the scaled-softmax denominator
     for free; unscaled softmax row-sums + diagonal via ones-vector matmuls
     (att = (1 - diag(softmax(S)))/scale). All accumulation in fp32 PSUM.
"""

import numpy as np

B, T, D = 16, 1024, 500
VOCAB, EMB = 50000, 300
NCORES = 8
BLOC = B // NCORES  # 2 examples per core
SCALE = float(np.sqrt(500.0))
NEG = 1e12

_CACHE = {}


def _build_bass():
    from contextlib import ExitStack

    import concourse.bacc as bacc
    import concourse.bass as bass
    import concourse.tile as tile
    from concourse import mybir
    from concourse.masks import make_identity

    f32 = mybir.dt.float32
    bf16 = mybir.dt.bfloat16
    i32 = mybir.dt.int32
    AF = mybir.ActivationFunctionType
    AX = mybir.AxisListType
    OP = mybir.AluOpType

    nc = bacc.Bacc(None, target_bir_lowering=False, debug=False)

    # ---- DRAM I/O ----
    words_d = nc.dram_tensor("words", [BLOC, 128, 8], i32, kind="ExternalInput")
    rest_d = nc.dram_tensor("rest", [BLOC, T, 200], bf16, kind="ExternalInput")
    pen_d = nc.dram_tensor("pen", [BLOC, T], bf16, kind="ExternalInput")
    emb_d = nc.dram_tensor("emb", [VOCAB, EMB], bf16, kind="ExternalInput")
    karr_d = nc.dram_tensor("karr", [126, 4, 500], bf16, kind="ExternalInput")
    qarr_d = nc.dram_tensor("qarr", [126, 4, 500], bf16, kind="ExternalInput")
    varr_d = nc.dram_tensor("varr", [126, 4, 500], bf16, kind="ExternalInput")
    wq_d = nc.dram_tensor("wq", [125, 8, 500], bf16, kind="ExternalInput")
    wc_d = nc.dram_tensor("wc", [125, 4, 500], bf16, kind="ExternalInput")
    wm_d = nc.dram_tensor("wm", [125, 12, 500], bf16, kind="ExternalInput")
    wk_d = nc.dram_tensor("wk", [125, 4], f32, kind="ExternalInput")
    bq_d = nc.dram_tensor("bq", [1, 500], f32, kind="ExternalInput")
    bc_d = nc.dram_tensor("bc", [1, 500], f32, kind="ExternalInput")
    bm_d = nc.dram_tensor("bm", [1, 500], f32, kind="ExternalInput")
    out_d = nc.dram_tensor("out", [BLOC, T, D], f32, kind="ExternalOutput")
    att_d = nc.dram_tensor("att", [BLOC, T], f32, kind="ExternalOutput")

    with tile.TileContext(nc) as tc, ExitStack() as ctx:
        sb = ctx.enter_context(tc.tile_pool(name="sb", bufs=1))
        ps = ctx.enter_context(tc.tile_pool(name="ps", bufs=1, space="PSUM"))

        # ---- constants / weights (loaded once) ----
        ident = sb.tile([128, 128], f32, name="ident", tag="ident", bufs=1)
        make_identity(nc, ident[:])
        ident_bf = sb.tile([128, 128], bf16, name="ident_bf", tag="ident_bf", bufs=1)
        make_identity(nc, ident_bf[:])
        ones_col = sb.tile([128, 1], bf16, name="ones_col", tag="ones_col", bufs=1)
        nc.vector.memset(ones_col[:], 1.0)
        ones_row = sb.tile([1, 128], f32, name="ones_row", tag="ones_row", bufs=1)
        nc.vector.memset(ones_row[:], 1.0)
        ones_row_bf = sb.tile([1, 128], bf16, name="ones_row_bf", tag="ones_row_bf", bufs=1)
        nc.vector.memset(ones_row_bf[:], 1.0)

        def emit_weights():
            karr = sb.tile([126, 4, 500], bf16, name="karr", tag="karr", bufs=1)
            nc.sync.dma_start(karr[:], karr_d[:])
            qarr = sb.tile([126, 4, 500], bf16, name="qarr", tag="qarr", bufs=1)
            nc.sync.dma_start(qarr[:], qarr_d[:])
            varr = sb.tile([126, 4, 500], bf16, name="varr", tag="varr", bufs=1)
            nc.sync.dma_start(varr[:], varr_d[:])
            wq = sb.tile([125, 8, 500], bf16, name="wq", tag="wq", bufs=1)
            nc.sync.dma_start(wq[:], wq_d[:])
            wc = sb.tile([125, 4, 500], bf16, name="wc", tag="wc", bufs=1)
            nc.sync.dma_start(wc[:], wc_d[:])
            wm = sb.tile([125, 12, 500], bf16, name="wm", tag="wm", bufs=1)
            nc.sync.dma_start(wm[:], wm_d[:])
            wk = sb.tile([125, 4], f32, name="wk", tag="wk", bufs=1)
            nc.sync.dma_start(wk[:], wk_d[:])
            bq = sb.tile([1, 500], f32, name="bq", tag="bq", bufs=1)
            nc.sync.dma_start(bq[:], bq_d[:])
            bc = sb.tile([1, 500], f32, name="bc", tag="bc", bufs=1)
            nc.sync.dma_start(bc[:], bc_d[:])
            bm = sb.tile([1, 500], f32, name="bm", tag="bm", bufs=1)
            nc.sync.dma_start(bm[:], bm_d[:])

            return karr, qarr, varr, wq, wc, wm, wk, bq, bc, bm

        # ================= woven per-example pipeline =================
        # Per example: prep (gather+transpose -> g_d, projections), then
        # attention halves. The serial q/t/k/c/m chain and the NEXT example's
        # gather are woven into the attention i-loop so the PE stream stays
        # dense (HAM stays warm). The attention output PSUM is released with
        # cheap copies (partA); normalization/m-multiply (partB) runs off the
        # critical path once the chain's m_b is ready.
        st_ = [dict() for _ in range(BLOC)]

        def emit_gd_init(e):
            s = st_[e]
            s["g_d"] = sb.tile([128, 4, T], bf16, name=f"g_d{e}", tag="g_d", bufs=2)
            nc.vector.memset(s["g_d"][96:128, :, :], 1.0)

        def emit_gather_init(e):
            s = st_[e]
            widx = sb.tile([128, 8], i32, name=f"widx{e}", tag="widx", bufs=2)
            nc.sync.dma_start(widx[:], words_d[e, :, :])
            s["widx"] = widx
            g_t = sb.tile([128, 8, 500], bf16, name=f"g_t{e}", tag="g_t", bufs=2)
            nc.sync.dma_start(
                g_t[:, :, EMB:500],
                rest_d[e].rearrange("(c p) f -> p c f", p=128),
            )
            s["g_t"] = g_t

        def emit_gather_chunk(e, tj):
            s = st_[e]
            t0 = tj * 128
            g_t = s["g_t"]
            nc.gpsimd.indirect_dma_start(
                out=g_t[:, tj, 0:EMB],
                out_offset=None,
                in_=emb_d[:],
                in_offset=bass.IndirectOffsetOnAxis(ap=s["widx"][:, tj : tj + 1], axis=0),
            )
            for c in range(4):
                tp = ps.tile([125, 128], bf16, name=f"tp{e}_{tj}_{c}", tag="pm", bufs=2)
                nc.tensor.transpose(tp[:], g_t[:, tj, c * 125 : (c + 1) * 125], ident_bf[:])
                nc.vector.tensor_copy(s["g_d"][0:125, c, t0 : t0 + 128], tp[:])

        def emit_proj_init(e):
            s = st_[e]
            s["keyT"] = sb.tile([125, 4, T], bf16, name=f"keyT{e}", tag="keyT", bufs=2)
            s["queryT"] = sb.tile([125, 4, T], bf16, name=f"queryT{e}", tag="queryT", bufs=2)
            s["value"] = sb.tile([128, 8, 501], bf16, name=f"value{e}", tag="value", bufs=2)

        def emit_proj_kq(e, h):
            s = st_[e]
            g_d = s["g_d"]
            for arr, dkey in ((karr, "keyT"), (qarr, "queryT")):
                for mc in range(4):
                    pp = ps.tile([125, 512], f32, name=f"pp{e}_{mc}_{h}", tag="pm", bufs=2)
                    for kk in range(4):
                        nc.tensor.matmul(
                            pp[:],
                            lhsT=arr[:, kk, mc * 125 : (mc + 1) * 125],
                            rhs=g_d[0:126, kk, h * 512 : (h + 1) * 512],
                            start=(kk == 0),
                            stop=(kk == 3),
                        )
                    nc.scalar.copy(s[dkey][:, mc, h * 512 : (h + 1) * 512], pp[:])

        def emit_value(e, tj):
            s = st_[e]
            g_d = s["g_d"]
            pv = ps.tile([128, 500], f32, name=f"pv{e}_{tj}", tag="pm", bufs=2)
            for kk in range(4):
                nc.tensor.matmul(
                    pv[:],
                    lhsT=g_d[0:126, kk, tj * 128 : (tj + 1) * 128],
                    rhs=varr[:, kk, :],
                    start=(kk == 0),
                    stop=(kk == 3),
                )
            nc.scalar.copy(s["value"][:, tj, 0:500], pv[:])
            nc.vector.memset(s["value"][:, tj, 500:501], 1.0)

        def emit_warmup(n):
            # keep the PE HAM activity monitor busy during DMA-bound stretches
            wu = ps.tile([128, 128], f32, name="wu", tag="rows", bufs=2)
            for _ in range(n):
                nc.tensor.matmul(wu[:], lhsT=ident_bf[:], rhs=ident_bf[:], start=True, stop=True)

        def chain_pieces(e):
            s = st_[e]
            g_d = s["g_d"]

            def p0():  # penalty bcast + subj max-pool (first half)
                s["pen_b"] = sb.tile([128, T], bf16, name=f"pen_b{e}", tag="pen_b", bufs=2)
                nc.sync.dma_start(s["pen_b"][:], pen_d[e : e + 1, :].to_broadcast([128, T]))
                s["subj_col"] = sb.tile([125, 4], bf16, name=f"subj_col{e}", tag="subj_col", bufs=2)
                for c in range(2):
                    tmp = sb.tile([125, T], bf16, name=f"tmpm{e}_{c}", tag="tmpm", bufs=2)
                    nc.vector.tensor_tensor(out=tmp[:], in0=g_d[0:125, c, :], in1=s["pen_b"][0:125, :], op=OP.min)
                    nc.vector.reduce_max(s["subj_col"][:, c : c + 1], tmp[:], axis=AX.X)

            def p1():  # subj max-pool (second half)
                for c in range(2, 4):
                    tmp = sb.tile([125, T], bf16, name=f"tmpm{e}_{c}", tag="tmpm", bufs=2)
                    nc.vector.tensor_tensor(out=tmp[:], in0=g_d[0:125, c, :], in1=s["pen_b"][0:125, :], op=OP.min)
                    nc.vector.reduce_max(s["subj_col"][:, c : c + 1], tmp[:], axis=AX.X)

            def p2():  # q = relu(so @ Wq + bq)
                q_ps = ps.tile([1, 512], f32, name=f"q_ps{e}", tag="rows", bufs=2)
                for kk in range(8):
                    nc.tensor.matmul(
                        q_ps[0:1, 0:500],
                        lhsT=s["subj_col"][:, (kk % 4) : (kk % 4) + 1],
                        rhs=wq[:, kk, :],
                        start=(kk == 0),
                        stop=(kk == 7),
                    )
                s["q_row"] = sb.tile([1, 500], f32, name=f"q_row{e}", tag="q_row", bufs=1)
                nc.vector.tensor_add(s["q_row"][:], q_ps[0:1, 0:500], bq[:])
                nc.scalar.activation(s["q_row"][:], s["q_row"][:], AF.Relu)

            def p3():  # q_row -> q_col
                s["q_col"] = sb.tile([125, 4], bf16, name=f"q_col{e}", tag="q_col", bufs=2)
                for c in range(4):
                    tpv = ps.tile([125, 1], f32, name=f"tpq{e}_{c}", tag="rows", bufs=2)
                    nc.tensor.transpose(tpv[:], s["q_row"][0:1, c * 125 : (c + 1) * 125], ident[0:1, 0:1])
                    nc.vector.tensor_copy(s["q_col"][:, c : c + 1], tpv[:])

            def p4():  # t = relu(q @ Wc + bc), t_row -> t_col
                t_ps = ps.tile([1, 512], f32, name=f"t_ps{e}", tag="rows", bufs=2)
                for kk in range(4):
                    nc.tensor.matmul(
                        t_ps[0:1, 0:500],
                        lhsT=s["q_col"][:, kk : kk + 1],
                        rhs=wc[:, kk, :],
                        start=(kk == 0),
                        stop=(kk == 3),
                    )
                t_row = sb.tile([1, 500], f32, name=f"t_row{e}", tag="t_row", bufs=1)
                nc.vector.tensor_add(t_row[:], t_ps[0:1, 0:500], bc[:])
                nc.scalar.activation(t_row[:], t_row[:], AF.Relu)
                s["t_col"] = sb.tile([125, 4], f32, name=f"t_col{e}", tag="t_col", bufs=2)
                for c in range(4):
                    tpt = ps.tile([125, 1], f32, name=f"tpt{e}_{c}", tag="rows", bufs=2)
                    nc.tensor.transpose(tpt[:], t_row[0:1, c * 125 : (c + 1) * 125], ident[0:1, 0:1])
                    nc.vector.tensor_copy(s["t_col"][:, c : c + 1], tpt[:])

            def p5():  # w_b = t*Wk ; k_logits
                wb_col = sb.tile([125, 4], bf16, name=f"wb_col{e}", tag="wb_col", bufs=2)
                nc.vector.tensor_mul(wb_col[:], s["t_col"][:], wk[:])
                s["klog"] = sb.tile([1, T], f32, name=f"klog{e}", tag="klog", bufs=1)
                for h in range(2):
                    kl_ps = ps.tile([1, 512], f32, name=f"klps{e}_{h}", tag="rows", bufs=2)
                    for kk in range(4):
                        nc.tensor.matmul(
                            kl_ps[0:1, :],
                            lhsT=wb_col[:, kk : kk + 1],
                            rhs=g_d[0:125, kk, h * 512 : (h + 1) * 512],
                            start=(kk == 0),
                            stop=(kk == 3),
                        )
                    nc.vector.tensor_copy(s["klog"][0:1, h * 512 : (h + 1) * 512], kl_ps[0:1, :])

            def p6():  # k = softmax(k_logits), to bf16
                klog = s["klog"]
                kmax = sb.tile([1, 1], f32, name=f"kmax{e}", tag="kmax", bufs=2)
                nc.vector.reduce_max(kmax[:], klog[:], axis=AX.X)
                negmax = sb.tile([1, 1], f32, name=f"negmax{e}", tag="negmax", bufs=2)
                nc.vector.tensor_scalar_mul(negmax[:], kmax[:], -1.0)
                ksum = sb.tile([1, 1], f32, name=f"ksum{e}", tag="ksum", bufs=2)
                nc.scalar.activation(klog[:], klog[:], AF.Exp, bias=negmax[:, 0:1], scale=1.0, accum_out=ksum[:])
                rksum = sb.tile([1, 1], f32, name=f"rksum{e}", tag="rksum", bufs=2)
                nc.vector.reciprocal(rksum[:], ksum[:])
                s["k_row"] = sb.tile([1, T], bf16, name=f"k_rowb{e}", tag="k_rowb", bufs=2)
                nc.vector.tensor_scalar_mul(s["k_row"][:], klog[:], rksum[:, 0:1])

            def p7():  # k broadcast via PE outer product
                s["k_b"] = sb.tile([128, T], bf16, name=f"k_b{e}", tag="k_b", bufs=2)
                for h in range(2):
                    kb_ps = ps.tile([128, 512], f32, name=f"kbps{e}_{h}", tag="pm", bufs=2)
                    nc.tensor.matmul(
                        kb_ps[:],
                        lhsT=ones_row_bf[:],
                        rhs=s["k_row"][0:1, h * 512 : (h + 1) * 512],
                        start=True,
                        stop=True,
                    )
                    nc.vector.tensor_copy(s["k_b"][:, h * 512 : (h + 1) * 512], kb_ps[:])

            def p8():  # c = sum_t k*g
                c32 = sb.tile([125, 4], f32, name=f"c32_{e}", tag="c32", bufs=2)
                for c in range(4):
                    tmpc = sb.tile([125, T], bf16, name=f"tmpc{e}_{c}", tag="tmpm", bufs=2)
                    nc.vector.tensor_mul(tmpc[:], g_d[0:125, c, :], s["k_b"][0:125, :])
                    nc.vector.reduce_sum(c32[:, c : c + 1], tmpc[:], axis=AX.X)
                s["c_col"] = sb.tile([125, 4], bf16, name=f"c_col{e}", tag="c_col", bufs=2)
                nc.vector.tensor_copy(s["c_col"][:], c32[:])

            def p9():  # m = relu([c, subj, subj] @ Wm + bm)
                m_ps = ps.tile([1, 512], f32, name=f"m_ps{e}", tag="rows", bufs=2)
                for kk in range(12):
                    col = s["c_col"] if kk < 4 else s["subj_col"]
                    nc.tensor.matmul(
                        m_ps[0:1, 0:500],
                        lhsT=col[:, (kk % 4) : (kk % 4) + 1],
                        rhs=wm[:, kk, :],
                        start=(kk == 0),
                        stop=(kk == 11),
                    )
                s["m_row"] = sb.tile([1, 500], f32, name=f"m_row{e}", tag="m_row", bufs=1)
                nc.vector.tensor_add(s["m_row"][:], m_ps[0:1, 0:500], bm[:])
                nc.scalar.activation(s["m_row"][:], s["m_row"][:], AF.Relu)

            def p10():  # m broadcast
                s["m_b"] = sb.tile([128, 500], f32, name=f"m_b{e}", tag="m_b", bufs=2)
                mb_ps = ps.tile([128, 500], f32, name=f"mbps{e}", tag="pm", bufs=2)
                nc.tensor.matmul(mb_ps[:], lhsT=ones_row[:], rhs=s["m_row"][0:1, :], start=True, stop=True)
                nc.vector.tensor_copy(s["m_b"][:], mb_ps[:])

            return [p0, p1, p2, p3, p4, p5, p6, p7, p8, p9, p10]

        def emit_attn_iter(e, h, i):
            s = st_[e]
            ts0 = h * 512
            st_ps = ps.tile([128, 512], f32, name=f"st{e}_{h}_{i}", tag="pm", bufs=2)
            for kk in range(4):
                nc.tensor.matmul(
                    st_ps[:],
                    lhsT=s["queryT"][:, kk, i * 128 : (i + 1) * 128],
                    rhs=s["keyT"][:, kk, ts0 : ts0 + 512],
                    start=(kk == 0),
                    stop=(kk == 3),
                )
            exp_s = sb.tile([128, 512], bf16, name=f"exps{e}_{h}_{i}", tag="exp_s", bufs=2)
            nc.scalar.activation(exp_s[:], st_ps[:], AF.Exp, scale=1.0 / SCALE)
            exp_u = sb.tile([128, 512], bf16, name=f"expu{e}_{h}_{i}", tag="exp_u", bufs=8)
            nc.scalar.activation(exp_u[:], st_ps[:], AF.Exp, scale=1.0)
            s.setdefault(("expu", h), []).append(exp_u)
            if i // 4 == h:
                off = (i % 4) * 128
                msk = sb.tile([128, 128], bf16, name=f"msk{e}_{h}_{i}", tag="msk", bufs=2)
                nc.vector.tensor_mul(msk[:], exp_u[:, off : off + 128], ident_bf[:])
                dg_ps = ps.tile([1, 512], f32, name=f"dgps{e}_{h}_{i}", tag="rows", bufs=2)
                nc.tensor.matmul(dg_ps[0:1, 0:128], lhsT=ones_col[:, 0:1], rhs=msk[:], start=True, stop=True)
                nc.vector.tensor_copy(s["diagr"][0:1, i * 128 : (i + 1) * 128], dg_ps[0:1, 0:128])
            for jj in range(4):
                nc.tensor.matmul(
                    s["out_ps"][:, jj, 0:501],
                    lhsT=exp_s[:, jj * 128 : (jj + 1) * 128],
                    rhs=s["value"][:, i, :],
                    start=(i == 0),
                    stop=(i == 7),
                )

        def emit_half_end(e, h):
            s = st_[e]
            ts0 = h * 512
            # unscaled softmax denominator: 8 quick accumulating matmuls
            se_ps = ps.tile([1, 512], f32, name=f"seps{e}_{h}", tag="rows", bufs=2)
            for i, exp_u in enumerate(s[("expu", h)]):
                nc.tensor.matmul(
                    se_ps[0:1, :], lhsT=ones_col[:, 0:1], rhs=exp_u[:], start=(i == 0), stop=(i == 7)
                )
            nc.vector.tensor_copy(s["sumexp"][0:1, ts0 : ts0 + 512], se_ps[0:1, :])
            # partA: release the attention output PSUM (per-bank copies)
            o_raw = sb.tile([128, 4, 501], f32, name=f"o_rawa{e}_{h}", tag="o_raw", bufs=3)
            for jj in range(4):
                nc.scalar.copy(o_raw[:, jj, :], s["out_ps"][:, jj, 0:501])
            s.setdefault("o_raw", []).append(o_raw)

        def emit_partB(e, js):
            s = st_[e]
            for j in js:
                o_raw = s["o_raw"][j // 4][:, j % 4, :]
                rec = sb.tile([128, 1], f32, name=f"rec{e}_{j}", tag="rec", bufs=2)
                nc.vector.reciprocal(rec[:], o_raw[:, 500:501])
                o_n = sb.tile([128, 500], f32, name=f"o_n{e}_{j}", tag="o_n", bufs=3)
                nc.vector.tensor_scalar_mul(o_n[:], o_raw[:, 0:500], rec[:, 0:1])
                nc.vector.tensor_mul(o_n[:], o_n[:], s["m_b"][:])
                nc.sync.dma_start(out_d[e, j * 128 : (j + 1) * 128, :], o_n[:])

        def emit_att_assembly(e):
            s = st_[e]
            sumexp_row, diag_row = s["sumexp"], s["diagr"]
            nc.vector.reciprocal(sumexp_row[:], sumexp_row[:])
            nc.vector.tensor_mul(diag_row[:], diag_row[:], sumexp_row[:])
            nc.vector.tensor_scalar(
                out=diag_row[:],
                in0=diag_row[:],
                scalar1=-1.0 / SCALE,
                scalar2=1.0 / SCALE,
                op0=mybir.AluOpType.mult,
                op1=mybir.AluOpType.add,
            )
            nc.sync.dma_start(att_d[e : e + 1, :], diag_row[:])

        # ---- pipeline ----
        emit_gd_init(0)
        emit_proj_init(0)
        emit_gather_init(0)
        karr, qarr, varr, wq, wc, wm, wk, bq, bc, bm = emit_weights()
        emit_warmup(24)
        for tj in range(8):
            emit_gather_chunk(0, tj)
            emit_value(0, tj)
            emit_warmup(8)
            if tj == 3:
                emit_proj_kq(0, 0)
        emit_proj_kq(0, 1)
        for e in range(BLOC):
            s = st_[e]
            nxt = e + 1 < BLOC
            s["sumexp"] = sb.tile([1, T], f32, name=f"sumexp{e}", tag="sumexp", bufs=1)
            s["diagr"] = sb.tile([1, T], f32, name=f"diagr{e}", tag="diagr", bufs=1)
            pieces = chain_pieces(e)
            # h = 0: weave chain pieces 0..7
            s["out_ps"] = ps.tile([128, 4, 512], f32, name=f"out_ps{e}_0", tag="po", bufs=1)
            for i in range(8):
                emit_attn_iter(e, 0, i)
                pieces[i]()
            emit_half_end(e, 0)
            # h = 1: weave remaining chain pieces, next example's prep, partB
            s["out_ps"] = ps.tile([128, 4, 512], f32, name=f"out_ps{e}_1", tag="po", bufs=1)
            if nxt:
                emit_gd_init(e + 1)
                emit_proj_init(e + 1)
                emit_gather_init(e + 1)
            for i in range(8):
                emit_attn_iter(e, 1, i)
                if i < 3:
                    pieces[8 + i]()
                if nxt:
                    emit_gather_chunk(e + 1, i)
                    emit_value(e + 1, i)
                    if i == 4:
                        emit_proj_kq(e + 1, 0)
                elif 3 <= i < 7:
                    emit_partB(e, [i - 3])
            emit_half_end(e, 1)
            if nxt:
                emit_proj_kq(e + 1, 1)
                emit_partB(e, range(8))
            else:
                emit_partB(e, range(4, 8))
            emit_att_assembly(e)

    nc.finalize()
    return nc


def _prep_host(inputs):
    """Host-side input prep: pack weights into SBUF-friendly layouts, gather the
    small embedding tables, build per-core input maps."""
    import ml_dtypes

    bf16 = ml_dtypes.bfloat16
    f = lambda k: np.asarray(inputs[k], dtype=np.float32)
    ii = lambda k: np.asarray(inputs[k], dtype=np.int64)

    words = ii("words")
    pos = ii("pos")
    ner = ii("ner")
    subj_pos = ii("subj_pos")
    obj_pos = ii("obj_pos")
    chunks = ii("chunks")
    on_path = ii("on_path")
    dep_feat = f("dep_feat")

    emb_w = f("emb_w")
    pos_w = f("pos_w")
    ner_w = f("ner_w")
    chunk_w = f("chunk_w")
    position_w = f("position_w")

    # rest200: host-gathered small-table features, cols 300..500 of g
    rest = np.concatenate(
        [
            pos_w[pos],                     # 35
            ner_w[ner],                     # 30
            chunk_w[chunks],                # 30
            position_w[subj_pos],           # 30
            position_w[obj_pos],            # 30
            on_path[..., None].astype(np.float32),  # 1
            dep_feat,                       # 44
        ],
        axis=2,
    ).astype(bf16)
    assert rest.shape == (B, T, 200)

    # penalty row for the masked max-pool: min(g, pen) == where(subj_pos!=0, -NEG, g)
    pen = np.where(subj_pos != 0, np.float32(-NEG), np.float32(3e38)).astype(bf16)

    def pack_kqv(w, b):
        # [126, 4, 500]: rows 0..124 of chunk c = W[125c : 125c+125]; row 125 of
        # chunk 0 = bias (multiplied by the all-ones row of g_d), else 0.
        arr = np.zeros((4, 126, 500), np.float32)
        w = np.asarray(w, np.float32)
        for c in range(4):
            arr[c, :125] = w[125 * c : 125 * (c + 1)]
        arr[0, 125] = np.asarray(b, np.float32)
        return np.ascontiguousarray(arr.transpose(1, 0, 2).astype(bf16))

    karr = pack_kqv(inputs["K_w"], inputs["K_b"])
    qarr = pack_kqv(inputs["Q_w"], inputs["Q_b"])
    varr = pack_kqv(inputs["V_w"], inputs["V_b"])

    def pack_rhs(w, nchunk):
        w = np.asarray(w, np.float32)
        return np.ascontiguousarray(
            w.reshape(nchunk, 125, 500).transpose(1, 0, 2).astype(bf16)
        )

    wq = pack_rhs(inputs["Wq_w"], 8)
    wc = pack_rhs(np.asarray(inputs["Wc_w"], np.float32)[:500], 4)
    wm = pack_rhs(inputs["Wm_w"], 12)
    wk = np.ascontiguousarray(
        np.asarray(inputs["Wk_w"], np.float32).reshape(4, 125).T
    )  # [125, 4], col c = Wk[125c:125c+125]
    bq = np.asarray(inputs["Wq_b"], np.float32).reshape(1, 500)
    bc = np.asarray(inputs["Wc_b"], np.float32).reshape(1, 500)
    bm = np.asarray(inputs["Wm_b"], np.float32).reshape(1, 500)

    shared = dict(
        emb=np.ascontiguousarray(emb_w.astype(bf16)),
        karr=karr, qarr=qarr, varr=varr,
        wq=wq, wc=wc, wm=wm, wk=wk, bq=bq, bc=bc, bm=bm,
    )
    in_maps = []
    for core in range(NCORES):
        s = slice(core * BLOC, (core + 1) * BLOC)
        m = dict(shared)
        m["words"] = np.ascontiguousarray(words[s].astype(np.int32).reshape(BLOC, 8, 128).transpose(0, 2, 1))
        m["rest"] = np.ascontiguousarray(rest[s])
        m["pen"] = np.ascontiguousarray(pen[s])
        in_maps.append(m)
    return in_maps


def _get_nc():
    if "nc" not in _CACHE:
        _CACHE["nc"] = _build_bass()
    return _CACHE["nc"]


def kernel(trace=False, **inputs):
    from concourse.bass_utils import run_bass_kernel_spmd

    nc = _get_nc()
    in_maps = _prep_host(inputs)
    res = run_bass_kernel_spmd(nc, in_maps, core_ids=list(range(NCORES)), trace=trace)
    results = res.results
    output = np.concatenate([r["out"] for r in results], axis=0)
    att = np.concatenate([r["att"] for r in results], axis=0)
    if trace:
        _CACHE["last_perf"] = res
    return output, att


# revision 39
# speedup vs baseline: 1.0397x; 1.0121x over previous
"""Trainium2 Bass kernel for nn_GCNClassifier (dense transformer w/ soft attention pooling).

Contract: kernel(**inputs) takes FULL unsharded inputs (as produced by
setup_inputs()) and returns the full output tuple (output[B,T,D], att[B,T]).
Internally: data-parallel over batch across 8 NeuronCores (2 examples/core).

Per-example device pipeline (bf16 matmul path, fp32 for the scalar chain):
  1. embedding gather (indirect DMA, words -> bf16 emb rows) into t-major g tiles;
     small-table features are host-gathered and streamed as one dense bf16 tensor.
  2. PE transposes g -> d-major layout g_d [500(+ones row), T], bf16.
  3. K/Q/V projections in bf16 (biases folded in via extra contraction row).
  4. subj max-pool / q / t / k-softmax / c / m chain (small fp32 vector-matmuls).
  5. attention in S^T layout: ST[s,t] = key[t].query[s]; exp on ACT (bf16 out);
     A@V with a ones column appended to V giving the scaled-softmax denominator
     for free; unscaled softmax row-sums + diagonal via ones-vector matmuls
     (att = (1 - diag(softmax(S)))/scale). All accumulation in fp32 PSUM.
"""

import numpy as np

B, T, D = 16, 1024, 500
VOCAB, EMB = 50000, 300
NCORES = 8
BLOC = B // NCORES  # 2 examples per core
SCALE = float(np.sqrt(500.0))
NEG = 1e12

_CACHE = {}


def _build_bass():
    from contextlib import ExitStack

    import concourse.bacc as bacc
    import concourse.bass as bass
    import concourse.tile as tile
    from concourse import mybir
    from concourse.masks import make_identity

    f32 = mybir.dt.float32
    bf16 = mybir.dt.bfloat16
    i32 = mybir.dt.int32
    AF = mybir.ActivationFunctionType
    AX = mybir.AxisListType
    OP = mybir.AluOpType

    nc = bacc.Bacc(None, target_bir_lowering=False, debug=False)

    # ---- DRAM I/O ----
    words_d = nc.dram_tensor("words", [BLOC, 128, 8], i32, kind="ExternalInput")
    rest_d = nc.dram_tensor("rest", [BLOC, T, 200], bf16, kind="ExternalInput")
    pen_d = nc.dram_tensor("pen", [BLOC, T], bf16, kind="ExternalInput")
    emb_d = nc.dram_tensor("emb", [VOCAB, EMB], bf16, kind="ExternalInput")
    karr_d = nc.dram_tensor("karr", [126, 4, 500], bf16, kind="ExternalInput")
    qarr_d = nc.dram_tensor("qarr", [126, 4, 500], bf16, kind="ExternalInput")
    varr_d = nc.dram_tensor("varr", [126, 4, 500], bf16, kind="ExternalInput")
    wq_d = nc.dram_tensor("wq", [125, 8, 500], bf16, kind="ExternalInput")
    wc_d = nc.dram_tensor("wc", [125, 4, 500], bf16, kind="ExternalInput")
    wm_d = nc.dram_tensor("wm", [125, 12, 500], bf16, kind="ExternalInput")
    wk_d = nc.dram_tensor("wk", [125, 4], f32, kind="ExternalInput")
    bq_d = nc.dram_tensor("bq", [1, 500], f32, kind="ExternalInput")
    bc_d = nc.dram_tensor("bc", [1, 500], f32, kind="ExternalInput")
    bm_d = nc.dram_tensor("bm", [1, 500], f32, kind="ExternalInput")
    out_d = nc.dram_tensor("out", [BLOC, T, D], f32, kind="ExternalOutput")
    att_d = nc.dram_tensor("att", [BLOC, T], f32, kind="ExternalOutput")

    with tile.TileContext(nc) as tc, ExitStack() as ctx:
        sb = ctx.enter_context(tc.tile_pool(name="sb", bufs=1))
        ps = ctx.enter_context(tc.tile_pool(name="ps", bufs=1, space="PSUM"))

        # ---- constants / weights (loaded once) ----
        ident = sb.tile([128, 128], f32, name="ident", tag="ident", bufs=1)
        make_identity(nc, ident[:])
        ident_bf = sb.tile([128, 128], bf16, name="ident_bf", tag="ident_bf", bufs=1)
        make_identity(nc, ident_bf[:])
        ones_col = sb.tile([128, 1], bf16, name="ones_col", tag="ones_col", bufs=1)
        nc.vector.memset(ones_col[:], 1.0)
        ones_row = sb.tile([1, 128], f32, name="ones_row", tag="ones_row", bufs=1)
        nc.vector.memset(ones_row[:], 1.0)
        ones_row_bf = sb.tile([1, 128], bf16, name="ones_row_bf", tag="ones_row_bf", bufs=1)
        nc.vector.memset(ones_row_bf[:], 1.0)

        def emit_weights():
            karr = sb.tile([126, 4, 500], bf16, name="karr", tag="karr", bufs=1)
            nc.sync.dma_start(karr[:], karr_d[:])
            qarr = sb.tile([126, 4, 500], bf16, name="qarr", tag="qarr", bufs=1)
            nc.sync.dma_start(qarr[:], qarr_d[:])
            varr = sb.tile([126, 4, 500], bf16, name="varr", tag="varr", bufs=1)
            nc.sync.dma_start(varr[:], varr_d[:])
            wq = sb.tile([125, 8, 500], bf16, name="wq", tag="wq", bufs=1)
            nc.sync.dma_start(wq[:], wq_d[:])
            wc = sb.tile([125, 4, 500], bf16, name="wc", tag="wc", bufs=1)
            nc.sync.dma_start(wc[:], wc_d[:])
            wm = sb.tile([125, 12, 500], bf16, name="wm", tag="wm", bufs=1)
            nc.sync.dma_start(wm[:], wm_d[:])
            wk = sb.tile([125, 4], f32, name="wk", tag="wk", bufs=1)
            nc.sync.dma_start(wk[:], wk_d[:])
            bq = sb.tile([1, 500], f32, name="bq", tag="bq", bufs=1)
            nc.sync.dma_start(bq[:], bq_d[:])
            bc = sb.tile([1, 500], f32, name="bc", tag="bc", bufs=1)
            nc.sync.dma_start(bc[:], bc_d[:])
            bm = sb.tile([1, 500], f32, name="bm", tag="bm", bufs=1)
            nc.sync.dma_start(bm[:], bm_d[:])

            return karr, qarr, varr, wq, wc, wm, wk, bq, bc, bm

        # ================= woven per-example pipeline =================
        # Per example: prep (gather+transpose -> g_d, projections), then
        # attention halves. The serial q/t/k/c/m chain and the NEXT example's
        # gather are woven into the attention i-loop so the PE stream stays
        # dense (HAM stays warm). The attention output PSUM is released with
        # cheap copies (partA); normalization/m-multiply (partB) runs off the
        # critical path once the chain's m_b is ready.
        st_ = [dict() for _ in range(BLOC)]

        def emit_gd_init(e):
            s = st_[e]
            s["g_d"] = sb.tile([128, 4, T], bf16, name=f"g_d{e}", tag="g_d", bufs=2)
            nc.vector.memset(s["g_d"][96:128, :, :], 1.0)

        def emit_gather_init(e):
            s = st_[e]
            widx = sb.tile([128, 8], i32, name=f"widx{e}", tag="widx", bufs=2)
            nc.sync.dma_start(widx[:], words_d[e, :, :])
            s["widx"] = widx
            g_t = sb.tile([128, 8, 500], bf16, name=f"g_t{e}", tag="g_t", bufs=2)
            nc.sync.dma_start(
                g_t[:, :, EMB:500],
                rest_d[e].rearrange("(c p) f -> p c f", p=128),
            )
            s["g_t"] = g_t

        def emit_gather_chunk(e, tj):
            s = st_[e]
            t0 = tj * 128
            g_t = s["g_t"]
            nc.gpsimd.indirect_dma_start(
                out=g_t[:, tj, 0:EMB],
                out_offset=None,
                in_=emb_d[:],
                in_offset=bass.IndirectOffsetOnAxis(ap=s["widx"][:, tj : tj + 1], axis=0),
            )
            for c in range(4):
                tp = ps.tile([125, 128], bf16, name=f"tp{e}_{tj}_{c}", tag="pm", bufs=2)
                nc.tensor.transpose(tp[:], g_t[:, tj, c * 125 : (c + 1) * 125], ident_bf[:])
                nc.vector.tensor_copy(s["g_d"][0:125, c, t0 : t0 + 128], tp[:])

        def emit_proj_init(e):
            s = st_[e]
            s["keyT"] = sb.tile([125, 4, T], bf16, name=f"keyT{e}", tag="keyT", bufs=2)
            s["queryT"] = sb.tile([125, 4, T], bf16, name=f"queryT{e}", tag="queryT", bufs=2)
            s["value"] = sb.tile([128, 8, 501], bf16, name=f"value{e}", tag="value", bufs=2)

        def emit_proj_kq(e, h):
            s = st_[e]
            g_d = s["g_d"]
            for arr, dkey in ((karr, "keyT"), (qarr, "queryT")):
                for mc in range(4):
                    pp = ps.tile([125, 512], f32, name=f"pp{e}_{mc}_{h}", tag="pm", bufs=2)
                    for kk in range(4):
                        nc.tensor.matmul(
                            pp[:],
                            lhsT=arr[:, kk, mc * 125 : (mc + 1) * 125],
                            rhs=g_d[0:126, kk, h * 512 : (h + 1) * 512],
                            start=(kk == 0),
                            stop=(kk == 3),
                        )
                    nc.scalar.copy(s[dkey][:, mc, h * 512 : (h + 1) * 512], pp[:])

        def emit_value(e, tj):
            s = st_[e]
            g_d = s["g_d"]
            pv = ps.tile([128, 500], f32, name=f"pv{e}_{tj}", tag="pm", bufs=2)
            for kk in range(4):
                nc.tensor.matmul(
                    pv[:],
                    lhsT=g_d[0:126, kk, tj * 128 : (tj + 1) * 128],
                    rhs=varr[:, kk, :],
                    start=(kk == 0),
                    stop=(kk == 3),
                )
            nc.scalar.copy(s["value"][:, tj, 0:500], pv[:])
            nc.vector.memset(s["value"][:, tj, 500:501], 1.0)

        def emit_warmup(n):
            # keep the PE HAM activity monitor busy during DMA-bound stretches
            wu = ps.tile([128, 128], f32, name="wu", tag="rows", bufs=2)
            for _ in range(n):
                nc.tensor.matmul(wu[:], lhsT=ident_bf[:], rhs=ident_bf[:], start=True, stop=True)

        def chain_pieces(e):
            s = st_[e]
            g_d = s["g_d"]

            def p0():  # penalty bcast + subj max-pool (first half)
                s["pen_b"] = sb.tile([128, T], bf16, name=f"pen_b{e}", tag="pen_b", bufs=2)
                nc.sync.dma_start(s["pen_b"][:], pen_d[e : e + 1, :].to_broadcast([128, T]))
                s["subj_col"] = sb.tile([125, 4], bf16, name=f"subj_col{e}", tag="subj_col", bufs=2)
                for c in range(2):
                    tmp = sb.tile([125, T], bf16, name=f"tmpm{e}_{c}", tag="tmpm", bufs=2)
                    nc.vector.tensor_tensor(out=tmp[:], in0=g_d[0:125, c, :], in1=s["pen_b"][0:125, :], op=OP.min)
                    nc.vector.reduce_max(s["subj_col"][:, c : c + 1], tmp[:], axis=AX.X)

            def p1():  # subj max-pool (second half)
                for c in range(2, 4):
                    tmp = sb.tile([125, T], bf16, name=f"tmpm{e}_{c}", tag="tmpm", bufs=2)
                    nc.vector.tensor_tensor(out=tmp[:], in0=g_d[0:125, c, :], in1=s["pen_b"][0:125, :], op=OP.min)
                    nc.vector.reduce_max(s["subj_col"][:, c : c + 1], tmp[:], axis=AX.X)

            def p2():  # q = relu(so @ Wq + bq)
                q_ps = ps.tile([1, 512], f32, name=f"q_ps{e}", tag="rows", bufs=2)
                for kk in range(8):
                    nc.tensor.matmul(
                        q_ps[0:1, 0:500],
                        lhsT=s["subj_col"][:, (kk % 4) : (kk % 4) + 1],
                        rhs=wq[:, kk, :],
                        start=(kk == 0),
                        stop=(kk == 7),
                    )
                s["q_row"] = sb.tile([1, 500], f32, name=f"q_row{e}", tag="q_row", bufs=1)
                nc.vector.tensor_add(s["q_row"][:], q_ps[0:1, 0:500], bq[:])
                nc.scalar.activation(s["q_row"][:], s["q_row"][:], AF.Relu)

            def p3():  # q_row -> q_col
                s["q_col"] = sb.tile([125, 4], bf16, name=f"q_col{e}", tag="q_col", bufs=2)
                for c in range(4):
                    tpv = ps.tile([125, 1], f32, name=f"tpq{e}_{c}", tag="rows", bufs=2)
                    nc.tensor.transpose(tpv[:], s["q_row"][0:1, c * 125 : (c + 1) * 125], ident[0:1, 0:1])
                    nc.vector.tensor_copy(s["q_col"][:, c : c + 1], tpv[:])

            def p4():  # t = relu(q @ Wc + bc), t_row -> t_col
                t_ps = ps.tile([1, 512], f32, name=f"t_ps{e}", tag="rows", bufs=2)
                for kk in range(4):
                    nc.tensor.matmul(
                        t_ps[0:1, 0:500],
                        lhsT=s["q_col"][:, kk : kk + 1],
                        rhs=wc[:, kk, :],
                        start=(kk == 0),
                        stop=(kk == 3),
                    )
                t_row = sb.tile([1, 500], f32, name=f"t_row{e}", tag="t_row", bufs=1)
                nc.vector.tensor_add(t_row[:], t_ps[0:1, 0:500], bc[:])
                nc.scalar.activation(t_row[:], t_row[:], AF.Relu)
                s["t_col"] = sb.tile([125, 4], f32, name=f"t_col{e}", tag="t_col", bufs=2)
                for c in range(4):
                    tpt = ps.tile([125, 1], f32, name=f"tpt{e}_{c}", tag="rows", bufs=2)
                    nc.tensor.transpose(tpt[:], t_row[0:1, c * 125 : (c + 1) * 125], ident[0:1, 0:1])
                    nc.vector.tensor_copy(s["t_col"][:, c : c + 1], tpt[:])

            def p5():  # w_b = t*Wk ; k_logits
                wb_col = sb.tile([125, 4], bf16, name=f"wb_col{e}", tag="wb_col", bufs=2)
                nc.vector.tensor_mul(wb_col[:], s["t_col"][:], wk[:])
                s["klog"] = sb.tile([1, T], f32, name=f"klog{e}", tag="klog", bufs=1)
                for h in range(2):
                    kl_ps = ps.tile([1, 512], f32, name=f"klps{e}_{h}", tag="rows", bufs=2)
                    for kk in range(4):
                        nc.tensor.matmul(
                            kl_ps[0:1, :],
                            lhsT=wb_col[:, kk : kk + 1],
                            rhs=g_d[0:125, kk, h * 512 : (h + 1) * 512],
                            start=(kk == 0),
                            stop=(kk == 3),
                        )
                    nc.vector.tensor_copy(s["klog"][0:1, h * 512 : (h + 1) * 512], kl_ps[0:1, :])

            def p6():  # k = softmax(k_logits), to bf16
                klog = s["klog"]
                kmax = sb.tile([1, 1], f32, name=f"kmax{e}", tag="kmax", bufs=2)
                nc.vector.reduce_max(kmax[:], klog[:], axis=AX.X)
                negmax = sb.tile([1, 1], f32, name=f"negmax{e}", tag="negmax", bufs=2)
                nc.vector.tensor_scalar_mul(negmax[:], kmax[:], -1.0)
                ksum = sb.tile([1, 1], f32, name=f"ksum{e}", tag="ksum", bufs=2)
                nc.scalar.activation(klog[:], klog[:], AF.Exp, bias=negmax[:, 0:1], scale=1.0, accum_out=ksum[:])
                rksum = sb.tile([1, 1], f32, name=f"rksum{e}", tag="rksum", bufs=2)
                nc.vector.reciprocal(rksum[:], ksum[:])
                s["k_row"] = sb.tile([1, T], bf16, name=f"k_rowb{e}", tag="k_rowb", bufs=2)
                nc.vector.tensor_scalar_mul(s["k_row"][:], klog[:], rksum[:, 0:1])

            def p7():  # k broadcast via PE outer product
                s["k_b"] = sb.tile([128, T], bf16, name=f"k_b{e}", tag="k_b", bufs=2)
                for h in range(2):
                    kb_ps = ps.tile([128, 512], f32, name=f"kbps{e}_{h}", tag="pm", bufs=2)
                    nc.tensor.matmul(
                        kb_ps[:],
                        lhsT=ones_row_bf[:],
                        rhs=s["k_row"][0:1, h * 512 : (h + 1) * 512],
                        start=True,
                        stop=True,
                    )
                    nc.vector.tensor_copy(s["k_b"][:, h * 512 : (h + 1) * 512], kb_ps[:])

            def p8():  # c = sum_t k*g
                c32 = sb.tile([125, 4], f32, name=f"c32_{e}", tag="c32", bufs=2)
                for c in range(4):
                    tmpc = sb.tile([125, T], bf16, name=f"tmpc{e}_{c}", tag="tmpm", bufs=2)
                    nc.vector.tensor_mul(tmpc[:], g_d[0:125, c, :], s["k_b"][0:125, :])
                    nc.vector.reduce_sum(c32[:, c : c + 1], tmpc[:], axis=AX.X)
                s["c_col"] = sb.tile([125, 4], bf16, name=f"c_col{e}", tag="c_col", bufs=2)
                nc.vector.tensor_copy(s["c_col"][:], c32[:])

            def p9():  # m = relu([c, subj, subj] @ Wm + bm)
                m_ps = ps.tile([1, 512], f32, name=f"m_ps{e}", tag="rows", bufs=2)
                for kk in range(12):
                    col = s["c_col"] if kk < 4 else s["subj_col"]
                    nc.tensor.matmul(
                        m_ps[0:1, 0:500],
                        lhsT=col[:, (kk % 4) : (kk % 4) + 1],
                        rhs=wm[:, kk, :],
                        start=(kk == 0),
                        stop=(kk == 11),
                    )
                s["m_row"] = sb.tile([1, 500], f32, name=f"m_row{e}", tag="m_row", bufs=1)
                nc.vector.tensor_add(s["m_row"][:], m_ps[0:1, 0:500], bm[:])
                nc.scalar.activation(s["m_row"][:], s["m_row"][:], AF.Relu)

            def p10():  # m broadcast
                s["m_b"] = sb.tile([128, 500], f32, name=f"m_b{e}", tag="m_b", bufs=2)
                mb_ps = ps.tile([128, 500], f32, name=f"mbps{e}", tag="pm", bufs=2)
                nc.tensor.matmul(mb_ps[:], lhsT=ones_row[:], rhs=s["m_row"][0:1, :], start=True, stop=True)
                nc.vector.tensor_copy(s["m_b"][:], mb_ps[:])

            return [p0, p1, p2, p3, p4, p5, p6, p7, p8, p9, p10]

        def emit_attn_st(e, h, i):
            s = st_[e]
            ts0 = h * 512
            st_ps = ps.tile([128, 512], f32, name=f"st{e}_{h}_{i}", tag="pm", bufs=2)
            for kk in range(4):
                nc.tensor.matmul(
                    st_ps[:],
                    lhsT=s["queryT"][:, kk, i * 128 : (i + 1) * 128],
                    rhs=s["keyT"][:, kk, ts0 : ts0 + 512],
                    start=(kk == 0),
                    stop=(kk == 3),
                )
            exp_s = sb.tile([128, 512], bf16, name=f"exps{e}_{h}_{i}", tag="exp_s", bufs=3)
            nc.scalar.activation(exp_s[:], st_ps[:], AF.Exp, scale=1.0 / SCALE)
            exp_u = sb.tile([128, 512], bf16, name=f"expu{e}_{h}_{i}", tag="exp_u", bufs=8)
            nc.scalar.activation(exp_u[:], st_ps[:], AF.Exp, scale=1.0)
            s.setdefault(("expu", h), []).append(exp_u)
            s[("exps", h, i)] = exp_s
            if i // 4 == h:
                off = (i % 4) * 128
                msk = sb.tile([128, 128], bf16, name=f"msk{e}_{h}_{i}", tag="msk", bufs=2)
                nc.vector.tensor_mul(msk[:], exp_u[:, off : off + 128], ident_bf[:])
                dg_ps = ps.tile([1, 512], f32, name=f"dgps{e}_{h}_{i}", tag="rows", bufs=2)
                nc.tensor.matmul(dg_ps[0:1, 0:128], lhsT=ones_col[:, 0:1], rhs=msk[:], start=True, stop=True)
                nc.vector.tensor_copy(s["diagr"][0:1, i * 128 : (i + 1) * 128], dg_ps[0:1, 0:128])

        def emit_attn_av(e, h, i):
            s = st_[e]
            exp_s = s.pop(("exps", h, i))
            for jj in range(4):
                nc.tensor.matmul(
                    s["out_ps"][:, jj, 0:501],
                    lhsT=exp_s[:, jj * 128 : (jj + 1) * 128],
                    rhs=s["value"][:, i, :],
                    start=(i == 0),
                    stop=(i == 7),
                )

        def emit_half_end(e, h):
            s = st_[e]
            ts0 = h * 512
            # unscaled softmax denominator: 8 quick accumulating matmuls
            se_ps = ps.tile([1, 512], f32, name=f"seps{e}_{h}", tag="rows", bufs=2)
            for i, exp_u in enumerate(s[("expu", h)]):
                nc.tensor.matmul(
                    se_ps[0:1, :], lhsT=ones_col[:, 0:1], rhs=exp_u[:], start=(i == 0), stop=(i == 7)
                )
            nc.vector.tensor_copy(s["sumexp"][0:1, ts0 : ts0 + 512], se_ps[0:1, :])
            # partA: release the attention output PSUM (per-bank copies)
            o_raw = sb.tile([128, 4, 501], f32, name=f"o_rawa{e}_{h}", tag="o_raw", bufs=3)
            for jj in range(4):
                nc.scalar.copy(o_raw[:, jj, :], s["out_ps"][:, jj, 0:501])
            s.setdefault("o_raw", []).append(o_raw)

        def emit_partB(e, js):
            s = st_[e]
            for j in js:
                o_raw = s["o_raw"][j // 4][:, j % 4, :]
                rec = sb.tile([128, 1], f32, name=f"rec{e}_{j}", tag="rec", bufs=2)
                nc.vector.reciprocal(rec[:], o_raw[:, 500:501])
                o_n = sb.tile([128, 500], f32, name=f"o_n{e}_{j}", tag="o_n", bufs=3)
                nc.scalar.activation(o_n[:], o_raw[:, 0:500], AF.Copy, scale=rec[:, 0:1])
                nc.vector.tensor_mul(o_n[:], o_n[:], s["m_b"][:])
                nc.sync.dma_start(out_d[e, j * 128 : (j + 1) * 128, :], o_n[:])

        def emit_att_assembly(e, h=None):
            s = st_[e]
            sumexp_row, diag_row = s["sumexp"], s["diagr"]
            nc.vector.reciprocal(sumexp_row[:], sumexp_row[:])
            nc.vector.tensor_mul(diag_row[:], diag_row[:], sumexp_row[:])
            nc.vector.tensor_scalar(
                out=diag_row[:],
                in0=diag_row[:],
                scalar1=-1.0 / SCALE,
                scalar2=1.0 / SCALE,
                op0=mybir.AluOpType.mult,
                op1=mybir.AluOpType.add,
            )
            nc.sync.dma_start(att_d[e : e + 1, :], diag_row[:])

        # ---- pipeline ----
        emit_gd_init(0)
        emit_proj_init(0)
        emit_gather_init(0)
        karr, qarr, varr, wq, wc, wm, wk, bq, bc, bm = emit_weights()
        emit_warmup(24)
        for tj in range(8):
            emit_gather_chunk(0, tj)
            emit_value(0, tj)
            emit_warmup(8)
            if tj == 3:
                emit_proj_kq(0, 0)
        emit_proj_kq(0, 1)
        for e in range(BLOC):
            s = st_[e]
            nxt = e + 1 < BLOC
            s["sumexp"] = sb.tile([1, T], f32, name=f"sumexp{e}", tag="sumexp", bufs=1)
            s["diagr"] = sb.tile([1, T], f32, name=f"diagr{e}", tag="diagr", bufs=1)
            pieces = chain_pieces(e)
            pieces[0]()  # subj part 1 (needs only g_d + pen)
            pieces[1]()  # subj part 2
            # h = 0: ST leads AV by one iteration; weave chain pieces 2..10
            s["out_ps"] = ps.tile([128, 4, 512], f32, name=f"out_ps{e}_0", tag="po", bufs=1)
            for i in range(8):
                emit_attn_st(e, 0, i)
                if i > 0:
                    emit_attn_av(e, 0, i - 1)
                if i < 7:
                    pieces[2 + i]()
            emit_attn_av(e, 0, 7)
            pieces[9]()
            emit_half_end(e, 0)
            # h = 1: weave last chain piece, next example's prep, partB
            s["out_ps"] = ps.tile([128, 4, 512], f32, name=f"out_ps{e}_1", tag="po", bufs=1)
            if nxt:
                emit_gd_init(e + 1)
                emit_proj_init(e + 1)
                emit_gather_init(e + 1)
            for i in range(8):
                emit_attn_st(e, 1, i)
                if i > 0:
                    emit_attn_av(e, 1, i - 1)
                if i == 0:
                    pieces[10]()
                if nxt:
                    emit_gather_chunk(e + 1, i)
                    emit_value(e + 1, i)
                    if i == 4:
                        emit_proj_kq(e + 1, 0)
                elif 1 <= i < 5:
                    emit_partB(e, [i - 1])
            emit_attn_av(e, 1, 7)
            emit_half_end(e, 1)
            emit_att_assembly(e, 1)
            if nxt:
                emit_proj_kq(e + 1, 1)
                emit_partB(e, range(8))
            else:
                emit_partB(e, range(4, 8))

    nc.finalize()
    return nc


def _prep_host(inputs):
    """Host-side input prep: pack weights into SBUF-friendly layouts, gather the
    small embedding tables, build per-core input maps."""
    import ml_dtypes

    bf16 = ml_dtypes.bfloat16
    f = lambda k: np.asarray(inputs[k], dtype=np.float32)
    ii = lambda k: np.asarray(inputs[k], dtype=np.int64)

    words = ii("words")
    pos = ii("pos")
    ner = ii("ner")
    subj_pos = ii("subj_pos")
    obj_pos = ii("obj_pos")
    chunks = ii("chunks")
    on_path = ii("on_path")
    dep_feat = f("dep_feat")

    emb_w = f("emb_w")
    pos_w = f("pos_w")
    ner_w = f("ner_w")
    chunk_w = f("chunk_w")
    position_w = f("position_w")

    # rest200: host-gathered small-table features, cols 300..500 of g
    rest = np.concatenate(
        [
            pos_w[pos],                     # 35
            ner_w[ner],                     # 30
            chunk_w[chunks],                # 30
            position_w[subj_pos],           # 30
            position_w[obj_pos],            # 30
            on_path[..., None].astype(np.float32),  # 1
            dep_feat,                       # 44
        ],
        axis=2,
    ).astype(bf16)
    assert rest.shape == (B, T, 200)

    # penalty row for the masked max-pool: min(g, pen) == where(subj_pos!=0, -NEG, g)
    pen = np.where(subj_pos != 0, np.float32(-NEG), np.float32(3e38)).astype(bf16)

    def pack_kqv(w, b):
        # [126, 4, 500]: rows 0..124 of chunk c = W[125c : 125c+125]; row 125 of
        # chunk 0 = bias (multiplied by the all-ones row of g_d), else 0.
        arr = np.zeros((4, 126, 500), np.float32)
        w = np.asarray(w, np.float32)
        for c in range(4):
            arr[c, :125] = w[125 * c : 125 * (c + 1)]
        arr[0, 125] = np.asarray(b, np.float32)
        return np.ascontiguousarray(arr.transpose(1, 0, 2).astype(bf16))

    karr = pack_kqv(inputs["K_w"], inputs["K_b"])
    qarr = pack_kqv(inputs["Q_w"], inputs["Q_b"])
    varr = pack_kqv(inputs["V_w"], inputs["V_b"])

    def pack_rhs(w, nchunk):
        w = np.asarray(w, np.float32)
        return np.ascontiguousarray(
            w.reshape(nchunk, 125, 500).transpose(1, 0, 2).astype(bf16)
        )

    wq = pack_rhs(inputs["Wq_w"], 8)
    wc = pack_rhs(np.asarray(inputs["Wc_w"], np.float32)[:500], 4)
    wm = pack_rhs(inputs["Wm_w"], 12)
    wk = np.ascontiguousarray(
        np.asarray(inputs["Wk_w"], np.float32).reshape(4, 125).T
    )  # [125, 4], col c = Wk[125c:125c+125]
    bq = np.asarray(inputs["Wq_b"], np.float32).reshape(1, 500)
    bc = np.asarray(inputs["Wc_b"], np.float32).reshape(1, 500)
    bm = np.asarray(inputs["Wm_b"], np.float32).reshape(1, 500)

    shared = dict(
        emb=np.ascontiguousarray(emb_w.astype(bf16)),
        karr=karr, qarr=qarr, varr=varr,
        wq=wq, wc=wc, wm=wm, wk=wk, bq=bq, bc=bc, bm=bm,
    )
    in_maps = []
    for core in range(NCORES):
        s = slice(core * BLOC, (core + 1) * BLOC)
        m = dict(shared)
        m["words"] = np.ascontiguousarray(words[s].astype(np.int32).reshape(BLOC, 8, 128).transpose(0, 2, 1))
        m["rest"] = np.ascontiguousarray(rest[s])
        m["pen"] = np.ascontiguousarray(pen[s])
        in_maps.append(m)
    return in_maps


def _get_nc():
    if "nc" not in _CACHE:
        _CACHE["nc"] = _build_bass()
    return _CACHE["nc"]


def kernel(trace=False, **inputs):
    from concourse.bass_utils import run_bass_kernel_spmd

    nc = _get_nc()
    in_maps = _prep_host(inputs)
    res = run_bass_kernel_spmd(nc, in_maps, core_ids=list(range(NCORES)), trace=trace)
    results = res.results
    output = np.concatenate([r["out"] for r in results], axis=0)
    att = np.concatenate([r["att"] for r in results], axis=0)
    if trace:
        _CACHE["last_perf"] = res
    return output, att


# revision 41
# speedup vs baseline: 1.0729x; 1.0319x over previous
"""Trainium2 Bass kernel for nn_GCNClassifier (dense transformer w/ soft attention pooling).

Contract: kernel(**inputs) takes FULL unsharded inputs (as produced by
setup_inputs()) and returns the full output tuple (output[B,T,D], att[B,T]).
Internally: data-parallel over batch across 8 NeuronCores (2 examples/core).

Per-example device pipeline (bf16 matmul path, fp32 for the scalar chain):
  1. embedding gather (indirect DMA, words -> bf16 emb rows) into t-major g tiles;
     small-table features are host-gathered and streamed as one dense bf16 tensor.
  2. PE transposes g -> d-major layout g_d [500(+ones row), T], bf16.
  3. K/Q/V projections in bf16 (biases folded in via extra contraction row).
  4. subj max-pool / q / t / k-softmax / c / m chain (small fp32 vector-matmuls).
  5. attention in S^T layout: ST[s,t] = key[t].query[s]; exp on ACT (bf16 out);
     A@V with a ones column appended to V giving the scaled-softmax denominator
     for free; unscaled softmax row-sums + diagonal via ones-vector matmuls
     (att = (1 - diag(softmax(S)))/scale). All accumulation in fp32 PSUM.
"""

import numpy as np

B, T, D = 16, 1024, 500
VOCAB, EMB = 50000, 300
NCORES = 8
BLOC = B // NCORES  # 2 examples per core
SCALE = float(np.sqrt(500.0))
NEG = 1e12

_CACHE = {}


def _build_bass():
    from contextlib import ExitStack

    import concourse.bacc as bacc
    import concourse.bass as bass
    import concourse.tile as tile
    from concourse import mybir
    from concourse.masks import make_identity

    f32 = mybir.dt.float32
    bf16 = mybir.dt.bfloat16
    i32 = mybir.dt.int32
    AF = mybir.ActivationFunctionType
    AX = mybir.AxisListType
    OP = mybir.AluOpType

    nc = bacc.Bacc(None, target_bir_lowering=False, debug=False)

    # ---- DRAM I/O ----
    words_d = nc.dram_tensor("words", [BLOC, 128, 8], i32, kind="ExternalInput")
    rest_d = nc.dram_tensor("rest", [BLOC, T, 200], bf16, kind="ExternalInput")
    pen_d = nc.dram_tensor("pen", [BLOC, T], bf16, kind="ExternalInput")
    emb_d = nc.dram_tensor("emb", [VOCAB, EMB], bf16, kind="ExternalInput")
    karr_d = nc.dram_tensor("karr", [126, 4, 500], bf16, kind="ExternalInput")
    qarr_d = nc.dram_tensor("qarr", [126, 4, 500], bf16, kind="ExternalInput")
    varr_d = nc.dram_tensor("varr", [126, 4, 500], bf16, kind="ExternalInput")
    wq_d = nc.dram_tensor("wq", [125, 8, 500], bf16, kind="ExternalInput")
    wc_d = nc.dram_tensor("wc", [125, 4, 500], bf16, kind="ExternalInput")
    wm_d = nc.dram_tensor("wm", [125, 12, 500], bf16, kind="ExternalInput")
    wk_d = nc.dram_tensor("wk", [125, 4], f32, kind="ExternalInput")
    bq_d = nc.dram_tensor("bq", [1, 500], f32, kind="ExternalInput")
    bc_d = nc.dram_tensor("bc", [1, 500], f32, kind="ExternalInput")
    bm_d = nc.dram_tensor("bm", [1, 500], f32, kind="ExternalInput")
    out_d = nc.dram_tensor("out", [BLOC, T, D], f32, kind="ExternalOutput")
    att_d = nc.dram_tensor("att", [2, BLOC, T], f32, kind="ExternalOutput")

    with tile.TileContext(nc) as tc, ExitStack() as ctx:
        sb = ctx.enter_context(tc.tile_pool(name="sb", bufs=1))
        ps = ctx.enter_context(tc.tile_pool(name="ps", bufs=1, space="PSUM"))

        # ---- constants / weights (loaded once) ----
        ident = sb.tile([128, 128], f32, name="ident", tag="ident", bufs=1)
        make_identity(nc, ident[:])
        ident_bf = sb.tile([128, 128], bf16, name="ident_bf", tag="ident_bf", bufs=1)
        make_identity(nc, ident_bf[:])
        ones_col = sb.tile([128, 1], bf16, name="ones_col", tag="ones_col", bufs=1)
        nc.vector.memset(ones_col[:], 1.0)
        ones_row = sb.tile([1, 128], f32, name="ones_row", tag="ones_row", bufs=1)
        nc.vector.memset(ones_row[:], 1.0)
        ones_row_bf = sb.tile([1, 128], bf16, name="ones_row_bf", tag="ones_row_bf", bufs=1)
        nc.vector.memset(ones_row_bf[:], 1.0)
        wu_src = sb.tile([128, 512], bf16, name="wu_src", tag="wu_src", bufs=1)
        nc.vector.memset(wu_src[:], 0.125)

        def emit_weights():
            karr = sb.tile([126, 4, 500], bf16, name="karr", tag="karr", bufs=1)
            nc.sync.dma_start(karr[:], karr_d[:])
            qarr = sb.tile([126, 4, 500], bf16, name="qarr", tag="qarr", bufs=1)
            nc.sync.dma_start(qarr[:], qarr_d[:])
            varr = sb.tile([126, 4, 500], bf16, name="varr", tag="varr", bufs=1)
            nc.sync.dma_start(varr[:], varr_d[:])
            wq = sb.tile([125, 8, 500], bf16, name="wq", tag="wq", bufs=1)
            nc.sync.dma_start(wq[:], wq_d[:])
            wc = sb.tile([125, 4, 500], bf16, name="wc", tag="wc", bufs=1)
            nc.sync.dma_start(wc[:], wc_d[:])
            wm = sb.tile([125, 12, 500], bf16, name="wm", tag="wm", bufs=1)
            nc.sync.dma_start(wm[:], wm_d[:])
            wk = sb.tile([125, 4], f32, name="wk", tag="wk", bufs=1)
            nc.sync.dma_start(wk[:], wk_d[:])
            bq = sb.tile([1, 500], f32, name="bq", tag="bq", bufs=1)
            nc.sync.dma_start(bq[:], bq_d[:])
            bc = sb.tile([1, 500], f32, name="bc", tag="bc", bufs=1)
            nc.sync.dma_start(bc[:], bc_d[:])
            bm = sb.tile([1, 500], f32, name="bm", tag="bm", bufs=1)
            nc.sync.dma_start(bm[:], bm_d[:])

            return karr, qarr, varr, wq, wc, wm, wk, bq, bc, bm

        # ================= woven per-example pipeline =================
        # Per example: prep (gather+transpose -> g_d, projections), then
        # attention halves. The serial q/t/k/c/m chain and the NEXT example's
        # gather are woven into the attention i-loop so the PE stream stays
        # dense (HAM stays warm). The attention output PSUM is released with
        # cheap copies (partA); normalization/m-multiply (partB) runs off the
        # critical path once the chain's m_b is ready.
        st_ = [dict() for _ in range(BLOC)]

        def emit_gd_init(e):
            s = st_[e]
            s["g_d"] = sb.tile([128, 4, T], bf16, name=f"g_d{e}", tag="g_d", bufs=2)
            nc.vector.memset(s["g_d"][96:128, :, :], 1.0)

        def emit_gather_init(e):
            s = st_[e]
            widx = sb.tile([128, 8], i32, name=f"widx{e}", tag="widx", bufs=2)
            nc.sync.dma_start(widx[:], words_d[e, :, :])
            s["widx"] = widx
            g_t = sb.tile([128, 8, 500], bf16, name=f"g_t{e}", tag="g_t", bufs=2)
            nc.sync.dma_start(
                g_t[:, :, EMB:500],
                rest_d[e].rearrange("(c p) f -> p c f", p=128),
            )
            s["g_t"] = g_t

        def emit_gather_chunk(e, tj):
            s = st_[e]
            t0 = tj * 128
            g_t = s["g_t"]
            nc.gpsimd.indirect_dma_start(
                out=g_t[:, tj, 0:EMB],
                out_offset=None,
                in_=emb_d[:],
                in_offset=bass.IndirectOffsetOnAxis(ap=s["widx"][:, tj : tj + 1], axis=0),
            )
            for c in range(4):
                tp = ps.tile([125, 128], bf16, name=f"tp{e}_{tj}_{c}", tag="pm", bufs=2)
                nc.tensor.transpose(tp[:], g_t[:, tj, c * 125 : (c + 1) * 125], ident_bf[:])
                nc.vector.tensor_copy(s["g_d"][0:125, c, t0 : t0 + 128], tp[:])

        def emit_proj_init(e):
            s = st_[e]
            s["keyT"] = sb.tile([125, 4, T], bf16, name=f"keyT{e}", tag="keyT", bufs=2)
            s["queryT"] = sb.tile([125, 4, T], bf16, name=f"queryT{e}", tag="queryT", bufs=2)
            s["value"] = sb.tile([128, 8, 501], bf16, name=f"value{e}", tag="value", bufs=2)

        def emit_proj_kq(e, h):
            s = st_[e]
            g_d = s["g_d"]
            for arr, dkey in ((karr, "keyT"), (qarr, "queryT")):
                for mc in range(4):
                    pp = ps.tile([125, 512], f32, name=f"pp{e}_{mc}_{h}", tag="pm", bufs=2)
                    for kk in range(4):
                        nc.tensor.matmul(
                            pp[:],
                            lhsT=arr[:, kk, mc * 125 : (mc + 1) * 125],
                            rhs=g_d[0:126, kk, h * 512 : (h + 1) * 512],
                            start=(kk == 0),
                            stop=(kk == 3),
                        )
                    nc.scalar.copy(s[dkey][:, mc, h * 512 : (h + 1) * 512], pp[:])

        def emit_value(e, tj):
            s = st_[e]
            g_d = s["g_d"]
            pv = ps.tile([128, 500], f32, name=f"pv{e}_{tj}", tag="pm", bufs=2)
            for kk in range(4):
                nc.tensor.matmul(
                    pv[:],
                    lhsT=g_d[0:126, kk, tj * 128 : (tj + 1) * 128],
                    rhs=varr[:, kk, :],
                    start=(kk == 0),
                    stop=(kk == 3),
                )
            nc.scalar.copy(s["value"][:, tj, 0:500], pv[:])
            nc.vector.memset(s["value"][:, tj, 500:501], 1.0)

        def emit_warmup(n):
            # keep the PE HAM activity monitor busy during DMA-bound stretches
            wu = ps.tile([128, 512], f32, name="wu", tag="rows", bufs=2)
            for _ in range(n):
                nc.tensor.matmul(wu[:], lhsT=ident_bf[:], rhs=wu_src[:], start=True, stop=True)

        def chain_pieces(e):
            s = st_[e]
            g_d = s["g_d"]

            def p0():  # penalty bcast + subj max-pool (first half)
                s["pen_b"] = sb.tile([128, T], bf16, name=f"pen_b{e}", tag="pen_b", bufs=2)
                nc.sync.dma_start(s["pen_b"][:], pen_d[e : e + 1, :].to_broadcast([128, T]))
                s["subj_col"] = sb.tile([125, 4], bf16, name=f"subj_col{e}", tag="subj_col", bufs=2)
                for c in range(2):
                    tmp = sb.tile([125, T], bf16, name=f"tmpm{e}_{c}", tag="tmpm", bufs=2)
                    nc.vector.tensor_tensor(out=tmp[:], in0=g_d[0:125, c, :], in1=s["pen_b"][0:125, :], op=OP.min)
                    nc.vector.reduce_max(s["subj_col"][:, c : c + 1], tmp[:], axis=AX.X)

            def p1():  # subj max-pool (second half)
                for c in range(2, 4):
                    tmp = sb.tile([125, T], bf16, name=f"tmpm{e}_{c}", tag="tmpm", bufs=2)
                    nc.vector.tensor_tensor(out=tmp[:], in0=g_d[0:125, c, :], in1=s["pen_b"][0:125, :], op=OP.min)
                    nc.vector.reduce_max(s["subj_col"][:, c : c + 1], tmp[:], axis=AX.X)

            def p2():  # q = relu(so @ Wq + bq)
                q_ps = ps.tile([1, 512], f32, name=f"q_ps{e}", tag="rows", bufs=2)
                for kk in range(8):
                    nc.tensor.matmul(
                        q_ps[0:1, 0:500],
                        lhsT=s["subj_col"][:, (kk % 4) : (kk % 4) + 1],
                        rhs=wq[:, kk, :],
                        start=(kk == 0),
                        stop=(kk == 7),
                    )
                s["q_row"] = sb.tile([1, 500], f32, name=f"q_row{e}", tag="q_row", bufs=1)
                nc.vector.tensor_add(s["q_row"][:], q_ps[0:1, 0:500], bq[:])
                nc.scalar.activation(s["q_row"][:], s["q_row"][:], AF.Relu)

            def p3():  # q_row -> q_col
                s["q_col"] = sb.tile([125, 4], bf16, name=f"q_col{e}", tag="q_col", bufs=2)
                for c in range(4):
                    tpv = ps.tile([125, 1], f32, name=f"tpq{e}_{c}", tag="rows", bufs=2)
                    nc.tensor.transpose(tpv[:], s["q_row"][0:1, c * 125 : (c + 1) * 125], ident[0:1, 0:1])
                    nc.vector.tensor_copy(s["q_col"][:, c : c + 1], tpv[:])

            def p4():  # t = relu(q @ Wc + bc), t_row -> t_col
                t_ps = ps.tile([1, 512], f32, name=f"t_ps{e}", tag="rows", bufs=2)
                for kk in range(4):
                    nc.tensor.matmul(
                        t_ps[0:1, 0:500],
                        lhsT=s["q_col"][:, kk : kk + 1],
                        rhs=wc[:, kk, :],
                        start=(kk == 0),
                        stop=(kk == 3),
                    )
                t_row = sb.tile([1, 500], f32, name=f"t_row{e}", tag="t_row", bufs=1)
                nc.vector.tensor_add(t_row[:], t_ps[0:1, 0:500], bc[:])
                nc.scalar.activation(t_row[:], t_row[:], AF.Relu)
                s["t_col"] = sb.tile([125, 4], f32, name=f"t_col{e}", tag="t_col", bufs=2)
                for c in range(4):
                    tpt = ps.tile([125, 1], f32, name=f"tpt{e}_{c}", tag="rows", bufs=2)
                    nc.tensor.transpose(tpt[:], t_row[0:1, c * 125 : (c + 1) * 125], ident[0:1, 0:1])
                    nc.vector.tensor_copy(s["t_col"][:, c : c + 1], tpt[:])

            def p5():  # w_b = t*Wk ; k_logits
                wb_col = sb.tile([125, 4], bf16, name=f"wb_col{e}", tag="wb_col", bufs=2)
                nc.vector.tensor_mul(wb_col[:], s["t_col"][:], wk[:])
                s["klog"] = sb.tile([1, T], f32, name=f"klog{e}", tag="klog", bufs=1)
                for h in range(2):
                    kl_ps = ps.tile([1, 512], f32, name=f"klps{e}_{h}", tag="rows", bufs=2)
                    for kk in range(4):
                        nc.tensor.matmul(
                            kl_ps[0:1, :],
                            lhsT=wb_col[:, kk : kk + 1],
                            rhs=g_d[0:125, kk, h * 512 : (h + 1) * 512],
                            start=(kk == 0),
                            stop=(kk == 3),
                        )
                    nc.vector.tensor_copy(s["klog"][0:1, h * 512 : (h + 1) * 512], kl_ps[0:1, :])

            def p6():  # k = softmax(k_logits), to bf16
                klog = s["klog"]
                kmax = sb.tile([1, 1], f32, name=f"kmax{e}", tag="kmax", bufs=2)
                nc.vector.reduce_max(kmax[:], klog[:], axis=AX.X)
                negmax = sb.tile([1, 1], f32, name=f"negmax{e}", tag="negmax", bufs=2)
                nc.vector.tensor_scalar_mul(negmax[:], kmax[:], -1.0)
                ksum = sb.tile([1, 1], f32, name=f"ksum{e}", tag="ksum", bufs=2)
                nc.scalar.activation(klog[:], klog[:], AF.Exp, bias=negmax[:, 0:1], scale=1.0, accum_out=ksum[:])
                rksum = sb.tile([1, 1], f32, name=f"rksum{e}", tag="rksum", bufs=2)
                nc.vector.reciprocal(rksum[:], ksum[:])
                s["k_row"] = sb.tile([1, T], bf16, name=f"k_rowb{e}", tag="k_rowb", bufs=2)
                nc.vector.tensor_scalar_mul(s["k_row"][:], klog[:], rksum[:, 0:1])

            def p7():  # k broadcast via PE outer product
                s["k_b"] = sb.tile([128, T], bf16, name=f"k_b{e}", tag="k_b", bufs=2)
                for h in range(2):
                    kb_ps = ps.tile([128, 512], f32, name=f"kbps{e}_{h}", tag="pm", bufs=2)
                    nc.tensor.matmul(
                        kb_ps[:],
                        lhsT=ones_row_bf[:],
                        rhs=s["k_row"][0:1, h * 512 : (h + 1) * 512],
                        start=True,
                        stop=True,
                    )
                    nc.vector.tensor_copy(s["k_b"][:, h * 512 : (h + 1) * 512], kb_ps[:])

            def p8():  # c = sum_t k*g
                c32 = sb.tile([125, 4], f32, name=f"c32_{e}", tag="c32", bufs=2)
                for c in range(4):
                    tmpc = sb.tile([125, T], bf16, name=f"tmpc{e}_{c}", tag="tmpm", bufs=2)
                    nc.vector.tensor_mul(tmpc[:], g_d[0:125, c, :], s["k_b"][0:125, :])
                    nc.vector.reduce_sum(c32[:, c : c + 1], tmpc[:], axis=AX.X)
                s["c_col"] = sb.tile([125, 4], bf16, name=f"c_col{e}", tag="c_col", bufs=2)
                nc.vector.tensor_copy(s["c_col"][:], c32[:])

            def p9():  # m = relu([c, subj, subj] @ Wm + bm)
                m_ps = ps.tile([1, 512], f32, name=f"m_ps{e}", tag="rows", bufs=2)
                for kk in range(12):
                    col = s["c_col"] if kk < 4 else s["subj_col"]
                    nc.tensor.matmul(
                        m_ps[0:1, 0:500],
                        lhsT=col[:, (kk % 4) : (kk % 4) + 1],
                        rhs=wm[:, kk, :],
                        start=(kk == 0),
                        stop=(kk == 11),
                    )
                s["m_row"] = sb.tile([1, 500], f32, name=f"m_row{e}", tag="m_row", bufs=1)
                nc.vector.tensor_add(s["m_row"][:], m_ps[0:1, 0:500], bm[:])
                nc.scalar.activation(s["m_row"][:], s["m_row"][:], AF.Relu)

            def p10():  # m broadcast
                s["m_b"] = sb.tile([128, 500], f32, name=f"m_b{e}", tag="m_b", bufs=2)
                mb_ps = ps.tile([128, 500], f32, name=f"mbps{e}", tag="pm", bufs=2)
                nc.tensor.matmul(mb_ps[:], lhsT=ones_row[:], rhs=s["m_row"][0:1, :], start=True, stop=True)
                nc.vector.tensor_copy(s["m_b"][:], mb_ps[:])

            return [p0, p1, p2, p3, p4, p5, p6, p7, p8, p9, p10]

        def emit_attn_st(e, h, i):
            s = st_[e]
            ts0 = h * 512
            st_ps = ps.tile([128, 512], f32, name=f"st{e}_{h}_{i}", tag="pm", bufs=2)
            for kk in range(4):
                nc.tensor.matmul(
                    st_ps[:],
                    lhsT=s["queryT"][:, kk, i * 128 : (i + 1) * 128],
                    rhs=s["keyT"][:, kk, ts0 : ts0 + 512],
                    start=(kk == 0),
                    stop=(kk == 3),
                )
            exp_s = sb.tile([128, 512], bf16, name=f"exps{e}_{h}_{i}", tag="exp_s", bufs=3)
            nc.scalar.activation(exp_s[:], st_ps[:], AF.Exp, scale=1.0 / SCALE)
            exp_u = sb.tile([128, 512], bf16, name=f"expu{e}_{h}_{i}", tag="exp_u", bufs=8)
            nc.scalar.activation(exp_u[:], st_ps[:], AF.Exp, scale=1.0)
            s.setdefault(("expu", h), []).append(exp_u)
            s[("exps", h, i)] = exp_s
            if i // 4 == h:
                off = (i % 4) * 128
                msk = sb.tile([128, 128], bf16, name=f"msk{e}_{h}_{i}", tag="msk", bufs=2)
                nc.vector.tensor_mul(msk[:], exp_u[:, off : off + 128], ident_bf[:])
                dg_ps = ps.tile([1, 512], f32, name=f"dgps{e}_{h}_{i}", tag="rows", bufs=2)
                nc.tensor.matmul(dg_ps[0:1, 0:128], lhsT=ones_col[:, 0:1], rhs=msk[:], start=True, stop=True)
                nc.vector.tensor_copy(s["diagr"][0:1, i * 128 : (i + 1) * 128], dg_ps[0:1, 0:128])

        def emit_attn_av(e, h, i):
            s = st_[e]
            exp_s = s.pop(("exps", h, i))
            for jj in range(4):
                nc.tensor.matmul(
                    s["out_ps"][:, jj, 0:501],
                    lhsT=exp_s[:, jj * 128 : (jj + 1) * 128],
                    rhs=s["value"][:, i, :],
                    start=(i == 0),
                    stop=(i == 7),
                )

        def emit_half_end(e, h):
            s = st_[e]
            ts0 = h * 512
            # unscaled softmax denominator: 8 quick accumulating matmuls
            se_ps = ps.tile([1, 512], f32, name=f"seps{e}_{h}", tag="rows", bufs=2)
            for i, exp_u in enumerate(s[("expu", h)]):
                nc.tensor.matmul(
                    se_ps[0:1, :], lhsT=ones_col[:, 0:1], rhs=exp_u[:], start=(i == 0), stop=(i == 7)
                )
            nc.vector.tensor_copy(s["sumexp"][0:1, ts0 : ts0 + 512], se_ps[0:1, :])
            nc.sync.dma_start(att_d[1, e : e + 1, ts0 : ts0 + 512], s["sumexp"][0:1, ts0 : ts0 + 512])
            nc.sync.dma_start(att_d[0, e : e + 1, ts0 : ts0 + 512], s["diagr"][0:1, ts0 : ts0 + 512])
            # partA: release the attention output PSUM (per-bank copies)
            o_raw = sb.tile([128, 4, 501], f32, name=f"o_rawa{e}_{h}", tag="o_raw", bufs=3)
            for jj in range(4):
                nc.scalar.copy(o_raw[:, jj, :], s["out_ps"][:, jj, 0:501])
            s.setdefault("o_raw", []).append(o_raw)

        def emit_partB(e, js):
            s = st_[e]
            for j in js:
                o_raw = s["o_raw"][j // 4][:, j % 4, :]
                rec = sb.tile([128, 1], f32, name=f"rec{e}_{j}", tag="rec", bufs=2)
                nc.vector.reciprocal(rec[:], o_raw[:, 500:501])
                o_n = sb.tile([128, 500], f32, name=f"o_n{e}_{j}", tag="o_n", bufs=3)
                nc.scalar.activation(o_n[:], o_raw[:, 0:500], AF.Copy, scale=rec[:, 0:1])
                if j % 2 == 0:
                    nc.vector.tensor_mul(o_n[:], o_n[:], s["m_b"][:])
                else:
                    nc.gpsimd.tensor_mul(o_n[:], o_n[:], s["m_b"][:])
                nc.sync.dma_start(out_d[e, j * 128 : (j + 1) * 128, :], o_n[:])

        # ---- pipeline ----
        emit_gd_init(0)
        emit_proj_init(0)
        emit_gather_init(0)
        karr, qarr, varr, wq, wc, wm, wk, bq, bc, bm = emit_weights()
        emit_warmup(10)
        for tj in range(8):
            emit_gather_chunk(0, tj)
            emit_value(0, tj)
            emit_warmup(3)
            if tj == 3:
                emit_proj_kq(0, 0)
        emit_proj_kq(0, 1)
        for e in range(BLOC):
            s = st_[e]
            nxt = e + 1 < BLOC
            s["sumexp"] = sb.tile([1, T], f32, name=f"sumexp{e}", tag="sumexp", bufs=1)
            s["diagr"] = sb.tile([1, T], f32, name=f"diagr{e}", tag="diagr", bufs=1)
            pieces = chain_pieces(e)
            pieces[0]()  # subj part 1 (needs only g_d + pen)
            pieces[1]()  # subj part 2
            # h = 0: ST leads AV by one iteration; weave chain pieces 2..10
            s["out_ps"] = ps.tile([128, 4, 512], f32, name=f"out_ps{e}_0", tag="po", bufs=1)
            for i in range(8):
                emit_attn_st(e, 0, i)
                if i > 0:
                    emit_attn_av(e, 0, i - 1)
                if i < 7:
                    pieces[2 + i]()
            emit_attn_av(e, 0, 7)
            pieces[9]()
            emit_half_end(e, 0)
            # h = 1: weave last chain piece, next example's prep, partB
            s["out_ps"] = ps.tile([128, 4, 512], f32, name=f"out_ps{e}_1", tag="po", bufs=1)
            if nxt:
                emit_gd_init(e + 1)
                emit_proj_init(e + 1)
                emit_gather_init(e + 1)
            for i in range(8):
                emit_attn_st(e, 1, i)
                if i > 0:
                    emit_attn_av(e, 1, i - 1)
                if i == 0:
                    pieces[10]()
                if nxt:
                    emit_gather_chunk(e + 1, i)
                    emit_value(e + 1, i)
                    if i == 4:
                        emit_proj_kq(e + 1, 0)
                elif 1 <= i < 5:
                    emit_partB(e, [i - 1])
            emit_attn_av(e, 1, 7)
            emit_half_end(e, 1)
            if nxt:
                emit_proj_kq(e + 1, 1)
                emit_partB(e, range(8))
            else:
                emit_partB(e, range(4, 8))

    nc.finalize()
    return nc


def _prep_host(inputs):
    """Host-side input prep: pack weights into SBUF-friendly layouts, gather the
    small embedding tables, build per-core input maps."""
    import ml_dtypes

    bf16 = ml_dtypes.bfloat16
    f = lambda k: np.asarray(inputs[k], dtype=np.float32)
    ii = lambda k: np.asarray(inputs[k], dtype=np.int64)

    words = ii("words")
    pos = ii("pos")
    ner = ii("ner")
    subj_pos = ii("subj_pos")
    obj_pos = ii("obj_pos")
    chunks = ii("chunks")
    on_path = ii("on_path")
    dep_feat = f("dep_feat")

    emb_w = f("emb_w")
    pos_w = f("pos_w")
    ner_w = f("ner_w")
    chunk_w = f("chunk_w")
    position_w = f("position_w")

    # rest200: host-gathered small-table features, cols 300..500 of g
    rest = np.concatenate(
        [
            pos_w[pos],                     # 35
            ner_w[ner],                     # 30
            chunk_w[chunks],                # 30
            position_w[subj_pos],           # 30
            position_w[obj_pos],            # 30
            on_path[..., None].astype(np.float32),  # 1
            dep_feat,                       # 44
        ],
        axis=2,
    ).astype(bf16)
    assert rest.shape == (B, T, 200)

    # penalty row for the masked max-pool: min(g, pen) == where(subj_pos!=0, -NEG, g)
    pen = np.where(subj_pos != 0, np.float32(-NEG), np.float32(3e38)).astype(bf16)

    def pack_kqv(w, b):
        # [126, 4, 500]: rows 0..124 of chunk c = W[125c : 125c+125]; row 125 of
        # chunk 0 = bias (multiplied by the all-ones row of g_d), else 0.
        arr = np.zeros((4, 126, 500), np.float32)
        w = np.asarray(w, np.float32)
        for c in range(4):
            arr[c, :125] = w[125 * c : 125 * (c + 1)]
        arr[0, 125] = np.asarray(b, np.float32)
        return np.ascontiguousarray(arr.transpose(1, 0, 2).astype(bf16))

    karr = pack_kqv(inputs["K_w"], inputs["K_b"])
    qarr = pack_kqv(inputs["Q_w"], inputs["Q_b"])
    varr = pack_kqv(inputs["V_w"], inputs["V_b"])

    def pack_rhs(w, nchunk):
        w = np.asarray(w, np.float32)
        return np.ascontiguousarray(
            w.reshape(nchunk, 125, 500).transpose(1, 0, 2).astype(bf16)
        )

    wq = pack_rhs(inputs["Wq_w"], 8)
    wc = pack_rhs(np.asarray(inputs["Wc_w"], np.float32)[:500], 4)
    wm = pack_rhs(inputs["Wm_w"], 12)
    wk = np.ascontiguousarray(
        np.asarray(inputs["Wk_w"], np.float32).reshape(4, 125).T
    )  # [125, 4], col c = Wk[125c:125c+125]
    bq = np.asarray(inputs["Wq_b"], np.float32).reshape(1, 500)
    bc = np.asarray(inputs["Wc_b"], np.float32).reshape(1, 500)
    bm = np.asarray(inputs["Wm_b"], np.float32).reshape(1, 500)

    shared = dict(
        emb=np.ascontiguousarray(emb_w.astype(bf16)),
        karr=karr, qarr=qarr, varr=varr,
        wq=wq, wc=wc, wm=wm, wk=wk, bq=bq, bc=bc, bm=bm,
    )
    in_maps = []
    for core in range(NCORES):
        s = slice(core * BLOC, (core + 1) * BLOC)
        m = dict(shared)
        m["words"] = np.ascontiguousarray(words[s].astype(np.int32).reshape(BLOC, 8, 128).transpose(0, 2, 1))
        m["rest"] = np.ascontiguousarray(rest[s])
        m["pen"] = np.ascontiguousarray(pen[s])
        in_maps.append(m)
    return in_maps


def _get_nc():
    if "nc" not in _CACHE:
        _CACHE["nc"] = _build_bass()
    return _CACHE["nc"]


def kernel(trace=False, **inputs):
    from concourse.bass_utils import run_bass_kernel_spmd

    nc = _get_nc()
    in_maps = _prep_host(inputs)
    res = run_bass_kernel_spmd(nc, in_maps, core_ids=list(range(NCORES)), trace=trace)
    results = res.results
    output = np.concatenate([r["out"] for r in results], axis=0)
    stats = np.concatenate([r["att"] for r in results], axis=1)  # [2, B, T]
    att = ((1.0 - stats[0] / stats[1]) / SCALE).astype(np.float32)
    if trace:
        _CACHE["last_perf"] = res
    return output, att


# revision 42
# speedup vs baseline: 1.1005x; 1.0257x over previous
"""Trainium2 Bass kernel for nn_GCNClassifier (dense transformer w/ soft attention pooling).

Contract: kernel(**inputs) takes FULL unsharded inputs (as produced by
setup_inputs()) and returns the full output tuple (output[B,T,D], att[B,T]).
Internally: data-parallel over batch across 8 NeuronCores (2 examples/core).

Per-example device pipeline (bf16 matmul path, fp32 for the scalar chain):
  1. embedding gather (indirect DMA, words -> bf16 emb rows) into t-major g tiles;
     small-table features are host-gathered and streamed as one dense bf16 tensor.
  2. PE transposes g -> d-major layout g_d [500(+ones row), T], bf16.
  3. K/Q/V projections in bf16 (biases folded in via extra contraction row).
  4. subj max-pool / q / t / k-softmax / c / m chain (small fp32 vector-matmuls).
  5. attention in S^T layout: ST[s,t] = key[t].query[s]; exp on ACT (bf16 out);
     A@V with a ones column appended to V giving the scaled-softmax denominator
     for free; unscaled softmax row-sums + diagonal via ones-vector matmuls
     (att = (1 - diag(softmax(S)))/scale). All accumulation in fp32 PSUM.
"""

import numpy as np

B, T, D = 16, 1024, 500
VOCAB, EMB = 50000, 300
NCORES = 8
BLOC = B // NCORES  # 2 examples per core
SCALE = float(np.sqrt(500.0))
NEG = 1e12

_CACHE = {}


def _build_bass():
    from contextlib import ExitStack

    import concourse.bacc as bacc
    import concourse.bass as bass
    import concourse.tile as tile
    from concourse import mybir
    from concourse.masks import make_identity

    f32 = mybir.dt.float32
    bf16 = mybir.dt.bfloat16
    i32 = mybir.dt.int32
    AF = mybir.ActivationFunctionType
    AX = mybir.AxisListType
    OP = mybir.AluOpType

    nc = bacc.Bacc(None, target_bir_lowering=False, debug=False)

    # ---- DRAM I/O ----
    words_d = nc.dram_tensor("words", [BLOC, 128, 8], i32, kind="ExternalInput")
    rest_d = nc.dram_tensor("rest", [BLOC, 200, T], bf16, kind="ExternalInput")
    pen_d = nc.dram_tensor("pen", [BLOC, T], bf16, kind="ExternalInput")
    emb_d = nc.dram_tensor("emb", [VOCAB, EMB], bf16, kind="ExternalInput")
    karr_d = nc.dram_tensor("karr", [126, 4, 500], bf16, kind="ExternalInput")
    qarr_d = nc.dram_tensor("qarr", [126, 4, 500], bf16, kind="ExternalInput")
    varr_d = nc.dram_tensor("varr", [126, 4, 500], bf16, kind="ExternalInput")
    wq_d = nc.dram_tensor("wq", [125, 8, 500], bf16, kind="ExternalInput")
    wc_d = nc.dram_tensor("wc", [125, 4, 500], bf16, kind="ExternalInput")
    wm_d = nc.dram_tensor("wm", [125, 12, 500], bf16, kind="ExternalInput")
    wk_d = nc.dram_tensor("wk", [125, 4], f32, kind="ExternalInput")
    bq_d = nc.dram_tensor("bq", [1, 500], f32, kind="ExternalInput")
    bc_d = nc.dram_tensor("bc", [1, 500], f32, kind="ExternalInput")
    bm_d = nc.dram_tensor("bm", [1, 500], f32, kind="ExternalInput")
    out_d = nc.dram_tensor("out", [BLOC, T, D], f32, kind="ExternalOutput")
    att_d = nc.dram_tensor("att", [2, BLOC, T], f32, kind="ExternalOutput")

    with tile.TileContext(nc) as tc, ExitStack() as ctx:
        sb = ctx.enter_context(tc.tile_pool(name="sb", bufs=1))
        ps = ctx.enter_context(tc.tile_pool(name="ps", bufs=1, space="PSUM"))

        # ---- constants / weights (loaded once) ----
        ident = sb.tile([128, 128], f32, name="ident", tag="ident", bufs=1)
        make_identity(nc, ident[:])
        ident_bf = sb.tile([128, 128], bf16, name="ident_bf", tag="ident_bf", bufs=1)
        make_identity(nc, ident_bf[:])
        ones_col = sb.tile([128, 1], bf16, name="ones_col", tag="ones_col", bufs=1)
        nc.vector.memset(ones_col[:], 1.0)
        ones_row = sb.tile([1, 128], f32, name="ones_row", tag="ones_row", bufs=1)
        nc.vector.memset(ones_row[:], 1.0)
        ones_row_bf = sb.tile([1, 128], bf16, name="ones_row_bf", tag="ones_row_bf", bufs=1)
        nc.vector.memset(ones_row_bf[:], 1.0)
        wu_src = sb.tile([128, 512], bf16, name="wu_src", tag="wu_src", bufs=1)
        nc.vector.memset(wu_src[:], 0.125)

        def emit_weights():
            karr = sb.tile([126, 4, 500], bf16, name="karr", tag="karr", bufs=1)
            nc.sync.dma_start(karr[:], karr_d[:])
            qarr = sb.tile([126, 4, 500], bf16, name="qarr", tag="qarr", bufs=1)
            nc.sync.dma_start(qarr[:], qarr_d[:])
            varr = sb.tile([126, 4, 500], bf16, name="varr", tag="varr", bufs=1)
            nc.sync.dma_start(varr[:], varr_d[:])
            wq = sb.tile([125, 8, 500], bf16, name="wq", tag="wq", bufs=1)
            nc.sync.dma_start(wq[:], wq_d[:])
            wc = sb.tile([125, 4, 500], bf16, name="wc", tag="wc", bufs=1)
            nc.sync.dma_start(wc[:], wc_d[:])
            wm = sb.tile([125, 12, 500], bf16, name="wm", tag="wm", bufs=1)
            nc.sync.dma_start(wm[:], wm_d[:])
            wk = sb.tile([125, 4], f32, name="wk", tag="wk", bufs=1)
            nc.sync.dma_start(wk[:], wk_d[:])
            bq = sb.tile([1, 500], f32, name="bq", tag="bq", bufs=1)
            nc.sync.dma_start(bq[:], bq_d[:])
            bc = sb.tile([1, 500], f32, name="bc", tag="bc", bufs=1)
            nc.sync.dma_start(bc[:], bc_d[:])
            bm = sb.tile([1, 500], f32, name="bm", tag="bm", bufs=1)
            nc.sync.dma_start(bm[:], bm_d[:])

            return karr, qarr, varr, wq, wc, wm, wk, bq, bc, bm

        # ================= woven per-example pipeline =================
        # Per example: prep (gather+transpose -> g_d, projections), then
        # attention halves. The serial q/t/k/c/m chain and the NEXT example's
        # gather are woven into the attention i-loop so the PE stream stays
        # dense (HAM stays warm). The attention output PSUM is released with
        # cheap copies (partA); normalization/m-multiply (partB) runs off the
        # critical path once the chain's m_b is ready.
        st_ = [dict() for _ in range(BLOC)]

        def emit_gd_init(e):
            s = st_[e]
            s["g_d"] = sb.tile([128, 4, T], bf16, name=f"g_d{e}", tag="g_d", bufs=2)
            nc.vector.memset(s["g_d"][96:128, :, :], 1.0)

        def emit_gather_init(e):
            s = st_[e]
            widx = sb.tile([128, 8], i32, name=f"widx{e}", tag="widx", bufs=2)
            nc.sync.dma_start(widx[:], words_d[e, :, :])
            s["widx"] = widx
            g_t = sb.tile([128, 8, EMB], bf16, name=f"g_t{e}", tag="g_t", bufs=2)
            s["g_t"] = g_t
            # feature rows 300..499 of g land directly in d-major layout:
            # chunk 2 rows 50..124 (d 300..374), chunk 3 rows 0..124 (d 375..499)
            nc.sync.dma_start(s["g_d"][50:125, 2, :], rest_d[e, 0:75, :])
            nc.sync.dma_start(s["g_d"][0:125, 3, :], rest_d[e, 75:200, :])

        def emit_gather_chunk(e, tj):
            s = st_[e]
            t0 = tj * 128
            g_t = s["g_t"]
            nc.gpsimd.indirect_dma_start(
                out=g_t[:, tj, 0:EMB],
                out_offset=None,
                in_=emb_d[:],
                in_offset=bass.IndirectOffsetOnAxis(ap=s["widx"][:, tj : tj + 1], axis=0),
            )
            for c in range(3):
                w = 125 if c < 2 else 50
                tp = ps.tile([125, 128], bf16, name=f"tp{e}_{tj}_{c}", tag="pm", bufs=2)
                nc.tensor.transpose(tp[0:w, :], g_t[:, tj, c * 125 : c * 125 + w], ident_bf[:])
                nc.vector.tensor_copy(s["g_d"][0:w, c, t0 : t0 + 128], tp[0:w, :])

        def emit_proj_init(e):
            s = st_[e]
            s["keyT"] = sb.tile([125, 4, T], bf16, name=f"keyT{e}", tag="keyT", bufs=2)
            s["queryT"] = sb.tile([125, 4, T], bf16, name=f"queryT{e}", tag="queryT", bufs=2)
            s["value"] = sb.tile([128, 8, 501], bf16, name=f"value{e}", tag="value", bufs=2)

        def emit_proj_kq(e, h):
            s = st_[e]
            g_d = s["g_d"]
            for arr, dkey in ((karr, "keyT"), (qarr, "queryT")):
                for mc in range(4):
                    pp = ps.tile([125, 512], f32, name=f"pp{e}_{mc}_{h}", tag="pm", bufs=2)
                    for kk in range(4):
                        nc.tensor.matmul(
                            pp[:],
                            lhsT=arr[:, kk, mc * 125 : (mc + 1) * 125],
                            rhs=g_d[0:126, kk, h * 512 : (h + 1) * 512],
                            start=(kk == 0),
                            stop=(kk == 3),
                        )
                    nc.scalar.copy(s[dkey][:, mc, h * 512 : (h + 1) * 512], pp[:])

        def emit_value(e, tj):
            s = st_[e]
            g_d = s["g_d"]
            pv = ps.tile([128, 500], f32, name=f"pv{e}_{tj}", tag="pm", bufs=2)
            for kk in range(4):
                nc.tensor.matmul(
                    pv[:],
                    lhsT=g_d[0:126, kk, tj * 128 : (tj + 1) * 128],
                    rhs=varr[:, kk, :],
                    start=(kk == 0),
                    stop=(kk == 3),
                )
            nc.scalar.copy(s["value"][:, tj, 0:500], pv[:])
            nc.vector.memset(s["value"][:, tj, 500:501], 1.0)

        def emit_warmup(n):
            # keep the PE HAM activity monitor busy during DMA-bound stretches
            wu = ps.tile([128, 512], f32, name="wu", tag="rows", bufs=2)
            for _ in range(n):
                nc.tensor.matmul(wu[:], lhsT=ident_bf[:], rhs=wu_src[:], start=True, stop=True)

        def chain_pieces(e):
            s = st_[e]
            g_d = s["g_d"]

            def p0():  # penalty bcast + subj max-pool (first half)
                s["pen_b"] = sb.tile([128, T], bf16, name=f"pen_b{e}", tag="pen_b", bufs=2)
                nc.sync.dma_start(s["pen_b"][:], pen_d[e : e + 1, :].to_broadcast([128, T]))
                s["subj_col"] = sb.tile([125, 4], bf16, name=f"subj_col{e}", tag="subj_col", bufs=2)
                for c in range(2):
                    tmp = sb.tile([125, T], bf16, name=f"tmpm{e}_{c}", tag="tmpm", bufs=2)
                    nc.vector.tensor_tensor(out=tmp[:], in0=g_d[0:125, c, :], in1=s["pen_b"][0:125, :], op=OP.min)
                    nc.vector.reduce_max(s["subj_col"][:, c : c + 1], tmp[:], axis=AX.X)

            def p1():  # subj max-pool (second half)
                for c in range(2, 4):
                    tmp = sb.tile([125, T], bf16, name=f"tmpm{e}_{c}", tag="tmpm", bufs=2)
                    nc.vector.tensor_tensor(out=tmp[:], in0=g_d[0:125, c, :], in1=s["pen_b"][0:125, :], op=OP.min)
                    nc.vector.reduce_max(s["subj_col"][:, c : c + 1], tmp[:], axis=AX.X)

            def p2():  # q = relu(so @ Wq + bq)
                q_ps = ps.tile([1, 512], f32, name=f"q_ps{e}", tag="rows", bufs=2)
                for kk in range(8):
                    nc.tensor.matmul(
                        q_ps[0:1, 0:500],
                        lhsT=s["subj_col"][:, (kk % 4) : (kk % 4) + 1],
                        rhs=wq[:, kk, :],
                        start=(kk == 0),
                        stop=(kk == 7),
                    )
                s["q_row"] = sb.tile([1, 500], f32, name=f"q_row{e}", tag="q_row", bufs=1)
                nc.vector.tensor_add(s["q_row"][:], q_ps[0:1, 0:500], bq[:])
                nc.scalar.activation(s["q_row"][:], s["q_row"][:], AF.Relu)

            def p3():  # q_row -> q_col
                s["q_col"] = sb.tile([125, 4], bf16, name=f"q_col{e}", tag="q_col", bufs=2)
                for c in range(4):
                    tpv = ps.tile([125, 1], f32, name=f"tpq{e}_{c}", tag="rows", bufs=2)
                    nc.tensor.transpose(tpv[:], s["q_row"][0:1, c * 125 : (c + 1) * 125], ident[0:1, 0:1])
                    nc.vector.tensor_copy(s["q_col"][:, c : c + 1], tpv[:])

            def p4():  # t = relu(q @ Wc + bc), t_row -> t_col
                t_ps = ps.tile([1, 512], f32, name=f"t_ps{e}", tag="rows", bufs=2)
                for kk in range(4):
                    nc.tensor.matmul(
                        t_ps[0:1, 0:500],
                        lhsT=s["q_col"][:, kk : kk + 1],
                        rhs=wc[:, kk, :],
                        start=(kk == 0),
                        stop=(kk == 3),
                    )
                t_row = sb.tile([1, 500], f32, name=f"t_row{e}", tag="t_row", bufs=1)
                nc.vector.tensor_add(t_row[:], t_ps[0:1, 0:500], bc[:])
                nc.scalar.activation(t_row[:], t_row[:], AF.Relu)
                s["t_col"] = sb.tile([125, 4], f32, name=f"t_col{e}", tag="t_col", bufs=2)
                for c in range(4):
                    tpt = ps.tile([125, 1], f32, name=f"tpt{e}_{c}", tag="rows", bufs=2)
                    nc.tensor.transpose(tpt[:], t_row[0:1, c * 125 : (c + 1) * 125], ident[0:1, 0:1])
                    nc.vector.tensor_copy(s["t_col"][:, c : c + 1], tpt[:])

            def p5():  # w_b = t*Wk ; k_logits
                wb_col = sb.tile([125, 4], bf16, name=f"wb_col{e}", tag="wb_col", bufs=2)
                nc.vector.tensor_mul(wb_col[:], s["t_col"][:], wk[:])
                s["klog"] = sb.tile([1, T], f32, name=f"klog{e}", tag="klog", bufs=1)
                for h in range(2):
                    kl_ps = ps.tile([1, 512], f32, name=f"klps{e}_{h}", tag="rows", bufs=2)
                    for kk in range(4):
                        nc.tensor.matmul(
                            kl_ps[0:1, :],
                            lhsT=wb_col[:, kk : kk + 1],
                            rhs=g_d[0:125, kk, h * 512 : (h + 1) * 512],
                            start=(kk == 0),
                            stop=(kk == 3),
                        )
                    nc.vector.tensor_copy(s["klog"][0:1, h * 512 : (h + 1) * 512], kl_ps[0:1, :])

            def p6():  # k = softmax(k_logits), to bf16
                klog = s["klog"]
                kmax = sb.tile([1, 1], f32, name=f"kmax{e}", tag="kmax", bufs=2)
                nc.vector.reduce_max(kmax[:], klog[:], axis=AX.X)
                negmax = sb.tile([1, 1], f32, name=f"negmax{e}", tag="negmax", bufs=2)
                nc.vector.tensor_scalar_mul(negmax[:], kmax[:], -1.0)
                ksum = sb.tile([1, 1], f32, name=f"ksum{e}", tag="ksum", bufs=2)
                nc.scalar.activation(klog[:], klog[:], AF.Exp, bias=negmax[:, 0:1], scale=1.0, accum_out=ksum[:])
                rksum = sb.tile([1, 1], f32, name=f"rksum{e}", tag="rksum", bufs=2)
                nc.vector.reciprocal(rksum[:], ksum[:])
                s["k_row"] = sb.tile([1, T], bf16, name=f"k_rowb{e}", tag="k_rowb", bufs=2)
                nc.vector.tensor_scalar_mul(s["k_row"][:], klog[:], rksum[:, 0:1])

            def p7():  # k broadcast via PE outer product
                s["k_b"] = sb.tile([128, T], bf16, name=f"k_b{e}", tag="k_b", bufs=2)
                for h in range(2):
                    kb_ps = ps.tile([128, 512], f32, name=f"kbps{e}_{h}", tag="pm", bufs=2)
                    nc.tensor.matmul(
                        kb_ps[:],
                        lhsT=ones_row_bf[:],
                        rhs=s["k_row"][0:1, h * 512 : (h + 1) * 512],
                        start=True,
                        stop=True,
                    )
                    nc.vector.tensor_copy(s["k_b"][:, h * 512 : (h + 1) * 512], kb_ps[:])

            def p8():  # c = sum_t k*g
                c32 = sb.tile([125, 4], f32, name=f"c32_{e}", tag="c32", bufs=2)
                for c in range(4):
                    tmpc = sb.tile([125, T], bf16, name=f"tmpc{e}_{c}", tag="tmpm", bufs=2)
                    nc.vector.tensor_mul(tmpc[:], g_d[0:125, c, :], s["k_b"][0:125, :])
                    nc.vector.reduce_sum(c32[:, c : c + 1], tmpc[:], axis=AX.X)
                s["c_col"] = sb.tile([125, 4], bf16, name=f"c_col{e}", tag="c_col", bufs=2)
                nc.vector.tensor_copy(s["c_col"][:], c32[:])

            def p9():  # m = relu([c, subj, subj] @ Wm + bm)
                m_ps = ps.tile([1, 512], f32, name=f"m_ps{e}", tag="rows", bufs=2)
                for kk in range(12):
                    col = s["c_col"] if kk < 4 else s["subj_col"]
                    nc.tensor.matmul(
                        m_ps[0:1, 0:500],
                        lhsT=col[:, (kk % 4) : (kk % 4) + 1],
                        rhs=wm[:, kk, :],
                        start=(kk == 0),
                        stop=(kk == 11),
                    )
                s["m_row"] = sb.tile([1, 500], f32, name=f"m_row{e}", tag="m_row", bufs=1)
                nc.vector.tensor_add(s["m_row"][:], m_ps[0:1, 0:500], bm[:])
                nc.scalar.activation(s["m_row"][:], s["m_row"][:], AF.Relu)

            def p10():  # m broadcast
                s["m_b"] = sb.tile([128, 500], f32, name=f"m_b{e}", tag="m_b", bufs=2)
                mb_ps = ps.tile([128, 500], f32, name=f"mbps{e}", tag="pm", bufs=2)
                nc.tensor.matmul(mb_ps[:], lhsT=ones_row[:], rhs=s["m_row"][0:1, :], start=True, stop=True)
                nc.vector.tensor_copy(s["m_b"][:], mb_ps[:])

            return [p0, p1, p2, p3, p4, p5, p6, p7, p8, p9, p10]

        def emit_attn_st(e, h, i):
            s = st_[e]
            ts0 = h * 512
            st_ps = ps.tile([128, 512], f32, name=f"st{e}_{h}_{i}", tag="pm", bufs=2)
            for kk in range(4):
                nc.tensor.matmul(
                    st_ps[:],
                    lhsT=s["queryT"][:, kk, i * 128 : (i + 1) * 128],
                    rhs=s["keyT"][:, kk, ts0 : ts0 + 512],
                    start=(kk == 0),
                    stop=(kk == 3),
                )
            exp_s = sb.tile([128, 512], bf16, name=f"exps{e}_{h}_{i}", tag="exp_s", bufs=3)
            nc.scalar.activation(exp_s[:], st_ps[:], AF.Exp, scale=1.0 / SCALE)
            exp_u = sb.tile([128, 512], bf16, name=f"expu{e}_{h}_{i}", tag="exp_u", bufs=8)
            nc.scalar.activation(exp_u[:], st_ps[:], AF.Exp, scale=1.0)
            s.setdefault(("expu", h), []).append(exp_u)
            s[("exps", h, i)] = exp_s
            if i // 4 == h:
                off = (i % 4) * 128
                msk = sb.tile([128, 128], bf16, name=f"msk{e}_{h}_{i}", tag="msk", bufs=2)
                nc.vector.tensor_mul(msk[:], exp_u[:, off : off + 128], ident_bf[:])
                dg_ps = ps.tile([1, 512], f32, name=f"dgps{e}_{h}_{i}", tag="rows", bufs=2)
                nc.tensor.matmul(dg_ps[0:1, 0:128], lhsT=ones_col[:, 0:1], rhs=msk[:], start=True, stop=True)
                nc.vector.tensor_copy(s["diagr"][0:1, i * 128 : (i + 1) * 128], dg_ps[0:1, 0:128])

        def emit_attn_av(e, h, i):
            s = st_[e]
            exp_s = s.pop(("exps", h, i))
            for jj in range(4):
                nc.tensor.matmul(
                    s["out_ps"][:, jj, 0:501],
                    lhsT=exp_s[:, jj * 128 : (jj + 1) * 128],
                    rhs=s["value"][:, i, :],
                    start=(i == 0),
                    stop=(i == 7),
                )

        def emit_half_end(e, h):
            s = st_[e]
            ts0 = h * 512
            # unscaled softmax denominator: 8 quick accumulating matmuls
            se_ps = ps.tile([1, 512], f32, name=f"seps{e}_{h}", tag="rows", bufs=2)
            for i, exp_u in enumerate(s[("expu", h)]):
                nc.tensor.matmul(
                    se_ps[0:1, :], lhsT=ones_col[:, 0:1], rhs=exp_u[:], start=(i == 0), stop=(i == 7)
                )
            nc.vector.tensor_copy(s["sumexp"][0:1, ts0 : ts0 + 512], se_ps[0:1, :])
            nc.sync.dma_start(att_d[1, e : e + 1, ts0 : ts0 + 512], s["sumexp"][0:1, ts0 : ts0 + 512])
            nc.sync.dma_start(att_d[0, e : e + 1, ts0 : ts0 + 512], s["diagr"][0:1, ts0 : ts0 + 512])
            # partA: release the attention output PSUM (per-bank copies)
            o_raw = sb.tile([128, 4, 501], f32, name=f"o_rawa{e}_{h}", tag="o_raw", bufs=3)
            for jj in range(4):
                nc.scalar.copy(o_raw[:, jj, :], s["out_ps"][:, jj, 0:501])
            s.setdefault("o_raw", []).append(o_raw)

        def emit_partB(e, js):
            s = st_[e]
            for j in js:
                o_raw = s["o_raw"][j // 4][:, j % 4, :]
                rec = sb.tile([128, 1], f32, name=f"rec{e}_{j}", tag="rec", bufs=2)
                nc.vector.reciprocal(rec[:], o_raw[:, 500:501])
                o_n = sb.tile([128, 500], f32, name=f"o_n{e}_{j}", tag="o_n", bufs=3)
                nc.scalar.activation(o_n[:], o_raw[:, 0:500], AF.Copy, scale=rec[:, 0:1])
                if j % 2 == 0:
                    nc.vector.tensor_mul(o_n[:], o_n[:], s["m_b"][:])
                else:
                    nc.gpsimd.tensor_mul(o_n[:], o_n[:], s["m_b"][:])
                nc.sync.dma_start(out_d[e, j * 128 : (j + 1) * 128, :], o_n[:])

        # ---- pipeline ----
        emit_gd_init(0)
        emit_proj_init(0)
        emit_gather_init(0)
        karr, qarr, varr, wq, wc, wm, wk, bq, bc, bm = emit_weights()
        emit_warmup(10)
        for tj in range(8):
            emit_gather_chunk(0, tj)
            emit_value(0, tj)
            emit_warmup(5)
            if tj == 3:
                emit_proj_kq(0, 0)
        emit_proj_kq(0, 1)
        for e in range(BLOC):
            s = st_[e]
            nxt = e + 1 < BLOC
            s["sumexp"] = sb.tile([1, T], f32, name=f"sumexp{e}", tag="sumexp", bufs=1)
            s["diagr"] = sb.tile([1, T], f32, name=f"diagr{e}", tag="diagr", bufs=1)
            pieces = chain_pieces(e)
            pieces[0]()  # subj part 1 (needs only g_d + pen)
            pieces[1]()  # subj part 2
            # h = 0: ST leads AV by one iteration; weave chain pieces 2..10
            s["out_ps"] = ps.tile([128, 4, 512], f32, name=f"out_ps{e}_0", tag="po", bufs=1)
            for i in range(8):
                emit_attn_st(e, 0, i)
                if i > 0:
                    emit_attn_av(e, 0, i - 1)
                if i < 7:
                    pieces[2 + i]()
            emit_attn_av(e, 0, 7)
            pieces[9]()
            emit_half_end(e, 0)
            # h = 1: weave last chain piece, next example's prep, partB
            s["out_ps"] = ps.tile([128, 4, 512], f32, name=f"out_ps{e}_1", tag="po", bufs=1)
            if nxt:
                emit_gd_init(e + 1)
                emit_proj_init(e + 1)
                emit_gather_init(e + 1)
            for i in range(8):
                emit_attn_st(e, 1, i)
                if i > 0:
                    emit_attn_av(e, 1, i - 1)
                if i == 0:
                    pieces[10]()
                if nxt:
                    emit_gather_chunk(e + 1, i)
                    emit_value(e + 1, i)
                    if i == 4:
                        emit_proj_kq(e + 1, 0)
                elif 1 <= i < 5:
                    emit_partB(e, [i - 1])
            emit_attn_av(e, 1, 7)
            emit_half_end(e, 1)
            if nxt:
                emit_proj_kq(e + 1, 1)
                emit_partB(e, range(8))
            else:
                emit_partB(e, range(4, 8))

    nc.finalize()
    return nc


def _prep_host(inputs):
    """Host-side input prep: pack weights into SBUF-friendly layouts, gather the
    small embedding tables, build per-core input maps."""
    import ml_dtypes

    bf16 = ml_dtypes.bfloat16
    f = lambda k: np.asarray(inputs[k], dtype=np.float32)
    ii = lambda k: np.asarray(inputs[k], dtype=np.int64)

    words = ii("words")
    pos = ii("pos")
    ner = ii("ner")
    subj_pos = ii("subj_pos")
    obj_pos = ii("obj_pos")
    chunks = ii("chunks")
    on_path = ii("on_path")
    dep_feat = f("dep_feat")

    emb_w = f("emb_w")
    pos_w = f("pos_w")
    ner_w = f("ner_w")
    chunk_w = f("chunk_w")
    position_w = f("position_w")

    # rest200: host-gathered small-table features, cols 300..500 of g
    rest = np.concatenate(
        [
            pos_w[pos],                     # 35
            ner_w[ner],                     # 30
            chunk_w[chunks],                # 30
            position_w[subj_pos],           # 30
            position_w[obj_pos],            # 30
            on_path[..., None].astype(np.float32),  # 1
            dep_feat,                       # 44
        ],
        axis=2,
    ).astype(bf16)
    assert rest.shape == (B, T, 200)
    rest = np.ascontiguousarray(rest.transpose(0, 2, 1))  # [B, 200, T]

    # penalty row for the masked max-pool: min(g, pen) == where(subj_pos!=0, -NEG, g)
    pen = np.where(subj_pos != 0, np.float32(-NEG), np.float32(3e38)).astype(bf16)

    def pack_kqv(w, b):
        # [126, 4, 500]: rows 0..124 of chunk c = W[125c : 125c+125]; row 125 of
        # chunk 0 = bias (multiplied by the all-ones row of g_d), else 0.
        arr = np.zeros((4, 126, 500), np.float32)
        w = np.asarray(w, np.float32)
        for c in range(4):
            arr[c, :125] = w[125 * c : 125 * (c + 1)]
        arr[0, 125] = np.asarray(b, np.float32)
        return np.ascontiguousarray(arr.transpose(1, 0, 2).astype(bf16))

    karr = pack_kqv(inputs["K_w"], inputs["K_b"])
    qarr = pack_kqv(inputs["Q_w"], inputs["Q_b"])
    varr = pack_kqv(inputs["V_w"], inputs["V_b"])

    def pack_rhs(w, nchunk):
        w = np.asarray(w, np.float32)
        return np.ascontiguousarray(
            w.reshape(nchunk, 125, 500).transpose(1, 0, 2).astype(bf16)
        )

    wq = pack_rhs(inputs["Wq_w"], 8)
    wc = pack_rhs(np.asarray(inputs["Wc_w"], np.float32)[:500], 4)
    wm = pack_rhs(inputs["Wm_w"], 12)
    wk = np.ascontiguousarray(
        np.asarray(inputs["Wk_w"], np.float32).reshape(4, 125).T
    )  # [125, 4], col c = Wk[125c:125c+125]
    bq = np.asarray(inputs["Wq_b"], np.float32).reshape(1, 500)
    bc = np.asarray(inputs["Wc_b"], np.float32).reshape(1, 500)
    bm = np.asarray(inputs["Wm_b"], np.float32).reshape(1, 500)

    shared = dict(
        emb=np.ascontiguousarray(emb_w.astype(bf16)),
        karr=karr, qarr=qarr, varr=varr,
        wq=wq, wc=wc, wm=wm, wk=wk, bq=bq, bc=bc, bm=bm,
    )
    in_maps = []
    for core in range(NCORES):
        s = slice(core * BLOC, (core + 1) * BLOC)
        m = dict(shared)
        m["words"] = np.ascontiguousarray(words[s].astype(np.int32).reshape(BLOC, 8, 128).transpose(0, 2, 1))
        m["rest"] = np.ascontiguousarray(rest[s])
        m["pen"] = np.ascontiguousarray(pen[s])
        in_maps.append(m)
    return in_maps


def _get_nc():
    if "nc" not in _CACHE:
        _CACHE["nc"] = _build_bass()
    return _CACHE["nc"]


def kernel(trace=False, **inputs):
    from concourse.bass_utils import run_bass_kernel_spmd

    nc = _get_nc()
    in_maps = _prep_host(inputs)
    res = run_bass_kernel_spmd(nc, in_maps, core_ids=list(range(NCORES)), trace=trace)
    results = res.results
    output = np.concatenate([r["out"] for r in results], axis=0)
    stats = np.concatenate([r["att"] for r in results], axis=1)  # [2, B, T]
    att = ((1.0 - stats[0] / stats[1]) / SCALE).astype(np.float32)
    if trace:
        _CACHE["last_perf"] = res
    return output, att
